# revision 2
# baseline (speedup 1.0000x reference)
"""Trainium2 Bass kernel for nn_Aligned_Feature_Aggregation.

Pipeline (B=2, N=8192, C=64, OUT=256, W=16, K=16):
  up1 = 3-NN inverse-distance interp of feature_1 (at xyz_1) onto xyz_2
  nf  = LeakyReLU(BN(conv1([up1; feature_2])))
  idx = 16-NN self KNN of xyz_2; WeightNet aggregation -> offsets [6, N]
  warped_{1,2} = xyz_2 + offsets; two 1-NN lookups against warped sets
  out = up1^T[idx1] + feature_2^T[idx2]   -> [B, N, 64]

Sharding: 8 cores = 2 batches x 4 query-quarters (2048 queries each).
Keys replicated per batch; AllGather (groups of 4) shares nf-derived fold
tables, up1 and the warped point sets.

Distance matrices are computed as Dt = 2*q.k - |k|^2 on the PE (fp32,
4-dim lifted contraction); bigger = nearer. Top-k per 1024-key chunk via
DVE max8 + max_index, then a mask-based merge (ties -> lowest index).
The [B,N,K,259]x[B,N,K,16] aggregation is folded through fc_w so only a
96-channel per-key table is gathered (R_all = fc3 . [k_xyz; nf]).
"""

import os
import numpy as np

import concourse.bass as bass
import concourse.mybir as mybir
import concourse.tile as tile_mod
from concourse.bass_utils import run_bass_kernel_spmd
from concourse.vector_clock import ScopedClock

F32 = mybir.dt.float32
U16 = mybir.dt.uint16
I16 = mybir.dt.int16
ALU = mybir.AluOpType
AF = mybir.ActivationFunctionType
AXX = mybir.AxisListType.X

B, N, C = 2, 8192, 64
OUT, W, K = 256, 16, 16
NQ = 2048            # queries per core
NT = 16              # query tiles of 128 per core
NCH = 8              # key chunks per tile
CH = 1024            # key chunk size
P = 128
BN_EPS = 1e-5
LEAKY = 0.1
NEG_BIG = -3.0e38
GROUPS = [[0, 1, 2, 3], [4, 5, 6, 7]]

DEBUG = bool(int(os.environ.get("BASS_KERNEL_DEBUG", "0")))


def _patch_tile_drain():
    """walrus in this env rejects >1 sem wait on the final SP drain; split."""
    if getattr(tile_mod.TileContext, "_drain_split_patched", False):
        return

    def _drain_and_barrier(self, tick_clock, wait_clock):
        nc = self.nc
        drain_inst = nc.sync.drain()
        wait_clock.add_sem_waits(
            drain_inst.ins, ScopedClock({None: tick_clock.global_clock})
        )
        si = drain_inst.ins.sync_info
        if si is not None and si.on_wait is not None and len(si.on_wait) > 1:
            waits = list(si.on_wait)
            si.on_wait = waits[:1]
            for w in waits[1:]:
                d2 = nc.sync.drain()
                d2.ins.sync_info = mybir.SyncInfo(on_wait=[w], on_update=[])
        nc.all_engine_barrier()
        assert self.sems is not None
        popped = nc._tile_sem_poison_stack.pop()
        assert popped is self._sem_poison
        nc.clear_and_free_semaphores(list(self.sems.allocated().values()))
        nc.all_engine_barrier()

    tile_mod.TileContext._drain_and_barrier = _drain_and_barrier
    tile_mod.TileContext._drain_split_patched = True


def build_program():
    _patch_tile_drain()
    nc = bass.Bass("TRN2", target_bir_lowering=False, debug=False)

    def din(name, shape):
        return nc.dram_tensor(name, shape, F32, kind="ExternalInput").ap()

    t = {}
    t["k1rows"] = din("k1rows", [3, N])
    t["k2rows"] = din("k2rows", [3, N])
    t["k1pt"] = din("k1pt", [P, 64, 3])
    t["k2pt"] = din("k2pt", [P, 64, 3])
    t["qrows"] = din("qrows", [3, NQ])
    t["qpt"] = din("qpt", [P, NT, 3])
    t["f1rows"] = din("f1rows", [C, N])
    t["f2loc"] = din("f2loc", [C, NQ])
    t["f2rows"] = din("f2rows", [C, N])
    t["convwt"] = din("convwt", [P, OUT])
    for nm in ("bng", "bnb", "bnm", "bnv"):
        t[nm] = din(nm, [P, 2])
    t["wn1t"] = din("wn1t", [3, 8])
    t["wn2t"] = din("wn2t", [8, 8])
    t["wn3t"] = din("wn3t", [8, W])
    t["wnb0"] = din("wnb0", [8, 1])
    t["wnb1"] = din("wnb1", [8, 1])
    t["wnb2"] = din("wnb2", [W, 1])
    t["rft"] = din("rft", [P, 192])
    t["rxt"] = din("rxt", [3, 96])
    t["fcb"] = din("fcb", [6, 1])
    t["selmat"] = din("selmat", [96, 6])
    t["wrep16"] = din("wrep16", [W, 96])
    t["sumsel"] = din("sumsel", [P, C])
    t["ident"] = din("ident", [P, P])
    t["gbase"] = din("gbase", [P, NCH * 8])

    t["out"] = nc.dram_tensor("out", [C, NQ], F32, kind="ExternalOutput").ap()
    if DEBUG:
        for nm, shp in [
            ("dbg_idx3", [P, NT * 3]),
            ("dbg_up1t", [C, NQ]),
            ("dbg_nf", [P, 2 * NQ]),
            ("dbg_idx16", [P, NT * 16]),
            ("dbg_off", [6, NQ]),
            ("dbg_idx12", [P, NT * 2]),
        ]:
            t[nm] = nc.dram_tensor(nm, shp, F32, kind="ExternalOutput").ap()

    t["k1aug"] = nc.dram_tensor("k1aug_d", [4, N], F32).ap()
    t["k2aug"] = nc.dram_tensor("k2aug_d", [4, N], F32).ap()
    t["w1aug"] = nc.dram_tensor("w1aug_d", [4, N], F32).ap()
    t["w2aug"] = nc.dram_tensor("w2aug_d", [4, N], F32).ap()
    t["mega_in"] = nc.dram_tensor("mega_in", [P, NQ], F32).ap()
    t["mega_out"] = nc.dram_tensor("mega_out", [4 * P, NQ], F32).ap()
    t["up1_in"] = nc.dram_tensor("up1_in", [C, NQ], F32).ap()
    t["up1_out"] = nc.dram_tensor("up1_out", [4 * C, NQ], F32).ap()
    t["warp_in"] = nc.dram_tensor("warp_in", [6, NQ], F32).ap()
    t["warp_out"] = nc.dram_tensor("warp_out", [24, NQ], F32).ap()

    with tile_mod.TileContext(nc) as tc:
        _build(nc, tc, t)
    _split_excess_waits(nc)
    return nc


def _split_excess_waits(nc, limit=1):
    """walrus rejects >2 sync waits per instruction: hoist extras onto NoOps."""
    for bbh in nc.bb_map.values():
        inner = bbh.bb
        insts = inner.instructions
        out = []
        changed = False
        for inst in insts:
            si = inst.sync_info
            waits = list(si.on_wait) if si is not None and si.on_wait else []
            if len(waits) > limit:
                excess, keep = waits[:-limit], waits[-limit:]
                for j in range(0, len(excess), limit):
                    nop = mybir.InstNoOp(
                        name=f"{inst.name}-ws{j}", ins=[], outs=[])
                    nop.engine = inst.engine
                    nop.sync_info = mybir.SyncInfo(
                        on_wait=excess[j:j + limit], on_update=[])
                    out.append(nop)
                si.on_wait = keep
                changed = True
            out.append(inst)
        if changed:
            inner.instructions = out


def _build(nc, tc, t):
    import contextlib
    ctx = contextlib.ExitStack()


    # ------------------------------------------------------------------
    # persistent SBUF
    # ------------------------------------------------------------------
    persist = ctx.enter_context(tc.tile_pool(name="persist", bufs=1))
    gtab = persist.tile([P, 2, 4112], F32, tag="gtab")  # rows 0:64 up1(AG), 64:128 f2
    qaug = persist.tile([4, NQ], F32, tag="qaug")
    outsb = persist.tile([C, NQ], F32, tag="outsb")

    consts = ctx.enter_context(tc.tile_pool(name="consts", bufs=1))

    def load_const(name, shape):
        s = consts.tile(shape, F32, tag="c_" + name)
        nc.sync.dma_start(s[:], t[name][:])
        return s

    convwt_lo = consts.tile([C, OUT], F32, tag="c_convlo")
    nc.sync.dma_start(convwt_lo[:], t["convwt"][0:C, :])
    convwt_hi = consts.tile([C, OUT], F32, tag="c_convhi")
    nc.sync.dma_start(convwt_hi[:], t["convwt"][C:P, :])
    wn1t_s = load_const("wn1t", [3, 8])
    wn2t_s = load_const("wn2t", [8, 8])
    wn3t_s = load_const("wn3t", [8, W])
    wnb0_s = load_const("wnb0", [8, 1])
    wnb1_s = load_const("wnb1", [8, 1])
    wnb2_s = load_const("wnb2", [W, 1])
    rft_s = load_const("rft", [P, 192])
    rxt_s = load_const("rxt", [3, 96])
    fcb_s = load_const("fcb", [6, 1])
    selmat_s = load_const("selmat", [96, 6])
    wrep16_s = load_const("wrep16", [W, 96])
    sumsel_s = load_const("sumsel", [P, C])
    ident_s = load_const("ident", [P, P])
    gbase_s = load_const("gbase", [P, NCH * 8])
    ones3_s = consts.tile([3, 1], F32, tag="c_ones3")
    nc.vector.memset(ones3_s[:], 1.0)
    ones64_s = consts.tile([1, C], F32, tag="c_ones64")
    nc.vector.memset(ones64_s[:], 1.0)
    qrows_s = load_const("qrows", [3, NQ])
    qpt_s = load_const("qpt", [P, NT, 3])

    # BN scale/bias: scale = g/sqrt(v+eps), bias = b - m*scale
    bn = consts.tile([P, 2, 4], F32, tag="bn")
    nc.sync.dma_start(bn[:, :, 0], t["bng"][:])
    nc.sync.dma_start(bn[:, :, 1], t["bnb"][:])
    nc.sync.dma_start(bn[:, :, 2], t["bnm"][:])
    nc.sync.dma_start(bn[:, :, 3], t["bnv"][:])
    bnsc = consts.tile([P, 2, 2], F32, tag="bnsc")
    tmpbn = consts.tile([P, 2], F32, tag="tmpbn")
    nc.vector.tensor_scalar_add(tmpbn[:], bn[:, :, 3], float(BN_EPS))
    nc.scalar.activation(tmpbn[:], tmpbn[:], AF.Sqrt)
    nc.vector.reciprocal(tmpbn[:], tmpbn[:])
    nc.vector.tensor_tensor(bnsc[:, :, 0], bn[:, :, 0], tmpbn[:], ALU.mult)
    nc.vector.tensor_tensor(tmpbn[:], bn[:, :, 2], bnsc[:, :, 0], ALU.mult)
    nc.vector.tensor_tensor(bnsc[:, :, 1], bn[:, :, 1], tmpbn[:], ALU.subtract)

    # qaug = [2*q; -1]
    nc.vector.memset(qaug[:], -1.0)
    nc.scalar.mul(qaug[0:3, :], qrows_s[:], 2.0)
    # qq6 = [q; q] for warped = offset + xyz2
    qq6 = persist.tile([6, NQ], F32, tag="qq6")
    nc.sync.dma_start(qq6[0:3, :], t["qrows"][:])
    nc.sync.dma_start(qq6[3:6, :], t["qrows"][:])

    # long-lived scratch pools
    mm_pool = ctx.enter_context(tc.tile_pool(name="mmp", bufs=3, space="PSUM"))
    ps_micro = ctx.enter_context(tc.tile_pool(name="psmicro", bufs=2, space="PSUM"))
    rhs_pool = ctx.enter_context(tc.tile_pool(name="rhsp", bufs=4))
    sel_pool = ctx.enter_context(tc.tile_pool(name="selp", bufs=2))
    tabp = ctx.enter_context(tc.tile_pool(name="tabp", bufs=2))

    # ------------------------------------------------------------------
    # helpers
    # ------------------------------------------------------------------
    def build_aug(pool, rows_src, pt_src_dram, dst_d, pt_sbuf=None):
        """aug table [4, N] in DRAM: rows 0:3 coords, row 3 = |k|^2."""
        nc.sync.dma_start(dst_d[0:3, :], rows_src)
        if pt_sbuf is None:
            ptt = pool.tile([P, 64, 3], F32, tag="augpt")
            nc.scalar.dma_start(ptt[:], pt_src_dram)
        else:
            ptt = pt_sbuf
        sq = pool.tile([P, 64, 3], F32, tag="augsq")
        nc.vector.tensor_tensor(sq[:], ptt[:], ptt[:], ALU.mult)
        nsq = pool.tile([P, 64], F32, tag="augn")
        nc.vector.tensor_reduce(nsq[:], sq[:], axis=AXX, op=ALU.add)
        # dst row3 col (g*128+p) <- nsq[p, g]
        nc.sync.dma_start(
            dst_d[3:4, :].rearrange("one (g p) -> one p g", p=P), nsq[:]
        )

    def dmat_select(ti, aug_d):
        """fp32 distance matmuls + per-chunk top8.  Returns (V, G) [P, 64] f32."""
        V = sel_pool.tile([P, NCH, 8], F32, tag="selV")
        Gu = sel_pool.tile([P, NCH, 8], U16, tag="selGu")
        lhs = qaug[:, ti * P:(ti + 1) * P]
        for cki in range(NCH):
            ps = mm_pool.tile([P, CH], F32, tag="dmat")
            rhs = rhs_pool.tile([4, CH], F32, tag="dmrhs")
            nc.scalar.dma_start(rhs[:], aug_d[:, cki * CH:(cki + 1) * CH])
            for h in range(2):
                nc.tensor.matmul(
                    ps[:, h * 512:(h + 1) * 512], lhs,
                    rhs[:, h * 512:(h + 1) * 512], start=True, stop=True,
                )
            nc.vector.max(out=V[:, cki, :], in_=ps[:])
            nc.vector.max_index(out=Gu[:, cki, :], in_max=V[:, cki, :], in_values=ps[:])
        Vf = V[:].rearrange("p a b -> p (a b)")
        G = sel_pool.tile([P, NCH * 8], F32, tag="selGf")
        nc.vector.tensor_copy(G[:], Gu[:].rearrange("p a b -> p (a b)"))
        nc.vector.tensor_tensor(G[:], G[:], gbase_s[:], ALU.add)
        return Vf, G

    def mask_extract(pool, Vf, G, ranks_ap, nk, tag):
        """idx[p, j] = G[p, pos(Vf == ranks[j])]; ties -> min index."""
        ncand = NCH * 8
        m = pool.tile([P, nk, ncand], mybir.dt.uint8, tag=tag + "m")
        nc.vector.tensor_tensor(
            m[:], Vf.unsqueeze(1).to_broadcast([P, nk, ncand]),
            ranks_ap.unsqueeze(2).to_broadcast([P, nk, ncand]), ALU.is_equal,
        )
        sel = pool.tile([P, nk, ncand], F32, tag=tag + "s")
        nc.vector.memset(sel[:], 65535.0)
        nc.vector.copy_predicated(
            sel[:], m[:], G[:].unsqueeze(1).to_broadcast([P, nk, ncand])
        )
        idx = pool.tile([P, nk], F32, tag=tag + "i")
        nc.vector.tensor_reduce(idx[:], sel[:], axis=AXX, op=ALU.min)
        return idx

    def transpose_pe(src_ap, m, tag):
        """[128, m<=128] -> PSUM [m, 128]"""
        ps = ps_micro.tile([m, P], F32, tag="psu")
        nc.tensor.matmul(ps[:], src_ap, ident_s[:], is_transpose=True)
        return ps

    def build_tab16(idxf16_ap, tag, second=None):
        """idxf16_ap [128, 16] f32 -> int16 ap_gather table [128, 128].

        Token i = q*16 + jj: unwrapped[i] = idxf16[q, jj] for every
        16-partition group. If `second` is given, groups 4..7 use it
        instead (per-group tables).
        """
        srcs = []
        for s_ap, stag in ((idxf16_ap, "a"), (second, "b")):
            if s_ap is None:
                srcs.append(None)
                continue
            tp = transpose_pe(s_ap, 16, tag + stag)
            tps = tabp.tile([16, P], F32, tag="ttps" + stag)
            nc.scalar.activation(tps[:], tp[:], AF.Copy)
            srcs.append(tps)
        tabf = tabp.tile([P, P], F32, tag="ttabf")
        for r in range(8):
            src = srcs[0] if (second is None or r < 4) else srcs[1]
            nc.gpsimd.dma_start(tabf[16 * r:16 * (r + 1), :], src[:])
        tab = tabp.tile([P, P], U16, tag="ttabi")
        nc.vector.tensor_copy(tab[:], tabf[:])
        return tab

    SW = 4112  # split-slice width (sentinel zero col at 4096 / 0)

    def gather8k(data2_ap, idx16f_ap, gpool, gtag, second=None):
        """Gather [128ch, 2048tok] from a [128, 2, 4112] split table.

        slice0 cols 0:4096 = keys 0:4096's data? actually keys 0:4095 at
        cols 0:4095, col 4096 = zero sentinel; slice1 col 0 = zero sentinel,
        cols 1:4096 = keys 4096:8191. idx16f in [0, 8192).
        """
        outs = []
        for which, idxsrc in (("a", idx16f_ap), ("b", second)):
            if idxsrc is None:
                idxsrc = idx16f_ap
            ia = tabp.tile([P, 16], F32, tag="gsa" + which)
            nc.vector.tensor_scalar_min(ia[:], idxsrc, 4096.0)
            ib = tabp.tile([P, 16], F32, tag="gsb" + which)
            nc.vector.tensor_scalar(
                ib[:], idxsrc, 4095.0, scalar2=0.0,
                op0=ALU.subtract, op1=ALU.max)
            outs.append((ia, ib))
        if second is None:
            tabA = build_tab16(outs[0][0][:], gtag + "A")
            tabB = build_tab16(outs[0][1][:], gtag + "B")
        else:
            tabA = build_tab16(outs[0][0][:], gtag + "A", second=outs[1][0][:])
            tabB = build_tab16(outs[0][1][:], gtag + "B", second=outs[1][1][:])
        gA = gpool.tile([P, 2048], F32, tag=gtag + "gA")
        gB = gpool.tile([P, 2048], F32, tag=gtag + "gB")
        for h in range(2):
            hsl = slice(h * 1024, (h + 1) * 1024)
            tsl = slice(h * 64, (h + 1) * 64)
            nc.gpsimd.indirect_copy(gA[:, hsl], data2_ap[:, 0, :], tabA[:, tsl], True)
            nc.gpsimd.indirect_copy(gB[:, hsl], data2_ap[:, 1, :], tabB[:, tsl], True)
        nc.vector.tensor_tensor(gA[:], gA[:], gB[:], ALU.add)
        return gA

    # ------------------------------------------------------------------
    # P0: key aug tables
    # ------------------------------------------------------------------
    with tc.tile_pool(name="p0", bufs=1) as p0:
        build_aug(p0, t["k1rows"][:], t["k1pt"][:], t["k1aug"])
        build_aug(p0, t["k2rows"][:], t["k2pt"][:], t["k2aug"])
    # f2 -> gtab rows 64:128 (split layout, sentinel cols zero)
    nc.vector.memset(gtab[:], 0.0)
    nc.sync.dma_start(gtab[C:P, 0, 0:4096], t["f2rows"][:, 0:4096])
    nc.sync.dma_start(gtab[C:P, 1, 1:4097], t["f2rows"][:, 4096:N])

    # ------------------------------------------------------------------
    # P1: D1 3-NN + upsample -> up1T
    # ------------------------------------------------------------------
    upool_cm = tc.tile_pool(name="upool", bufs=1)
    upool = upool_cm.__enter__()
    up1T = upool.tile([C, NQ], F32, tag="up1T")
    with tc.tile_pool(name="p1big", bufs=1) as p1big, \
         tc.tile_pool(name="p1s", bufs=1) as p1s:
        # split gather table [128, 2, 4112]: rows 0:3 xyz1, 64:128 feat1
        p1sb = p1big.tile([P, 2, SW], F32, tag="p1sb")
        nc.vector.memset(p1sb[:], 0.0)
        nc.sync.dma_start(p1sb[0:3, 0, 0:4096], t["k1rows"][:, 0:4096])
        nc.sync.dma_start(p1sb[0:3, 1, 1:4097], t["k1rows"][:, 4096:N])
        nc.sync.dma_start(p1sb[C:P, 0, 0:4096], t["f1rows"][:, 0:4096])
        nc.sync.dma_start(p1sb[C:P, 1, 1:4097], t["f1rows"][:, 4096:N])
        for ti in range(NT):
            Vf, G = dmat_select(ti, t["k1aug"])
            T8 = p1s.tile([P, 8], F32, tag="d1t8")
            nc.vector.max(out=T8[:], in_=Vf)
            idx3 = mask_extract(p1s, Vf, G, T8[:, 0:3], 3, "d1x")
            if DEBUG:
                nc.sync.dma_start(t["dbg_idx3"][:, ti * 3:(ti + 1) * 3], idx3[:])
            idx16p = p1s.tile([P, 16], F32, tag="d1pad")
            nc.vector.tensor_copy(idx16p[:, 0:3], idx3[:])
            nc.vector.tensor_copy(
                idx16p[:, 3:16], idx3[:, 0:1].to_broadcast([P, 13]))
            gout = gather8k(p1sb[:], idx16p[:], p1s, "p1g")
            # token f = q*16 + jj (jj<3 used). rows 0:3 xyz1, 3:67 feat1
            gx = p1s.tile([3, P, 3], F32, tag="upg")
            nc.vector.tensor_tensor(
                gx[:], gout[0:3, :].rearrange("c (q jj) -> c q jj", jj=16)[:, :, 0:3],
                qrows_s[:, ti * P:(ti + 1) * P].unsqueeze(2).to_broadcast([3, P, 3]),
                ALU.subtract,
            )
            nc.vector.tensor_tensor(gx[:], gx[:], gx[:], ALU.mult)
            gxs = p1s.tile([3, P * 3], F32, tag="upgs")
            nc.vector.tensor_copy(gxs[:], gx[:].rearrange("c q jj -> c (q jj)"))
            psd = ps_micro.tile([1, P * 3], F32, tag="psu")
            nc.tensor.matmul(psd[:], ones3_s[:], gxs[:], start=True, stop=True)
            dist = p1s.tile([1, P, 3], F32, tag="updist")
            nc.scalar.activation(
                dist[:].rearrange("one q jj -> one (q jj)"), psd[:], AF.Sqrt)
            nc.vector.tensor_scalar_max(dist[:], dist[:], 1e-10)
            w3 = p1s.tile([1, P, 3], F32, tag="upw")
            nc.vector.reciprocal(w3[:], dist[:])
            wsum = p1s.tile([1, P], F32, tag="upws")
            nc.vector.tensor_reduce(wsum[:], w3[:], axis=AXX, op=ALU.add)
            nc.vector.reciprocal(wsum[:], wsum[:])
            nc.vector.tensor_tensor(
                w3[:], w3[:], wsum[:].unsqueeze(2).to_broadcast([1, P, 3]), ALU.mult)
            # replicate wn to 64 partitions via PE, padded to 16 jj (zeros)
            wn16 = p1s.tile([1, P, 16], F32, tag="wn16")
            nc.vector.memset(wn16[:], 0.0)
            nc.vector.tensor_copy(wn16[:, :, 0:3], w3[:])
            wrep = p1s.tile([P, 2048], F32, tag="uwrep")
            for ck in range(4):
                psx = ps_micro.tile([C, 512], F32, tag="psu")
                nc.tensor.matmul(
                    psx[:], ones64_s[:],
                    wn16[:].rearrange("one q jj -> one (q jj)")[:, ck * 512:(ck + 1) * 512],
                    start=True, stop=True,
                )
                nc.scalar.activation(wrep[C:P, ck * 512:(ck + 1) * 512], psx[:], AF.Copy)
            wf = p1s.tile([P, 2048], F32, tag="upwf")
            nc.vector.tensor_tensor(wf[C:P, :], gout[C:P, :], wrep[C:P, :], ALU.mult)
            nc.vector.tensor_reduce(
                up1T[:, ti * P:(ti + 1) * P],
                wf[C:P, :].rearrange("c (q jj) -> c q jj", jj=16),
                axis=AXX, op=ALU.add,
            )

    if DEBUG:
        nc.sync.dma_start(t["dbg_up1t"][:], up1T[:])

    # ------------------------------------------------------------------
    # P2: conv/BN/LeakyReLU -> nf; fold tables; AllGathers
    # ------------------------------------------------------------------
    if True:
        with tc.tile_pool(name="p2", bufs=1) as p2:
            f2loc_s = p2.tile([C, NQ], F32, tag="f2loc")
            nc.sync.dma_start(f2loc_s[:], t["f2loc"][:])
            nfsb = p2.tile([P, 2, NQ], F32, tag="nfsb")
            for h in range(2):
                for ck in range(4):
                    sl = slice(ck * 512, (ck + 1) * 512)
                    ps = ps_micro.tile([P, 512], F32, tag="psu")
                    nc.tensor.matmul(ps[:], convwt_lo[:, h * P:(h + 1) * P],
                                     up1T[:, sl], start=True, stop=False)
                    nc.tensor.matmul(ps[:], convwt_hi[:, h * P:(h + 1) * P],
                                     f2loc_s[:, sl], start=False, stop=True)
                    nc.scalar.activation(
                        nfsb[:, h, sl], ps[:], AF.Copy,
                        bias=0.0, scale=bnsc[:, h, 0:1],
                    )
                    # Copy ignores AP bias; add bias then LeakyReLU = max(x, 0.1x)
                    nc.vector.tensor_tensor(
                        nfsb[:, h, sl], nfsb[:, h, sl],
                        bnsc[:, h, 1:2].to_broadcast([P, 512]), ALU.add)
                    nc.vector.scalar_tensor_tensor(
                        nfsb[:, h, sl], nfsb[:, h, sl], LEAKY, nfsb[:, h, sl],
                        op0=ALU.mult, op1=ALU.max)
            if DEBUG:
                nc.sync.dma_start(
                    t["dbg_nf"][:], nfsb[:].rearrange("p a b -> p (a b)"))

            mega_loc = p2.tile([P, NQ], F32, tag="megaloc")
            nc.vector.memset(mega_loc[:], 0.0)
            for ck in range(4):
                sl = slice(ck * 512, (ck + 1) * 512)
                ps = ps_micro.tile([96, 512], F32, tag="psu")
                nc.tensor.matmul(ps[:], rft_s[:, 0:96], nfsb[:, 0, sl],
                                 start=True, stop=False)
                nc.tensor.matmul(ps[:], rft_s[:, 96:192], nfsb[:, 1, sl],
                                 start=False, stop=False)
                nc.tensor.matmul(ps[:], rxt_s[:], qrows_s[:, sl],
                                 start=False, stop=True)
                nc.scalar.activation(mega_loc[0:96, sl], ps[:], AF.Copy)
                ps2 = ps_micro.tile([8, 512], F32, tag="psu")
                nc.tensor.matmul(ps2[:], wn1t_s[:], qrows_s[:, sl],
                                 start=True, stop=True)
                nc.scalar.activation(mega_loc[96:104, sl], ps2[:], AF.Copy)

            nc.sync.dma_start(t["mega_in"][:], mega_loc[:])
            nc.gpsimd.collective_compute(
                "AllGather", ALU.bypass, replica_groups=GROUPS,
                ins=[t["mega_in"][:].opt()], outs=[t["mega_out"][:].opt()],
            )
            nc.sync.dma_start(t["up1_in"][:], up1T[:])
            nc.gpsimd.collective_compute(
                "AllGather", ALU.bypass, replica_groups=GROUPS,
                ins=[t["up1_in"][:].opt()], outs=[t["up1_out"][:].opt()],
            )
            for blk in range(4):
                s, off = blk // 2, (blk % 2) * NQ + (blk // 2)
                nc.sync.dma_start(
                    gtab[0:C, s, off:off + NQ],
                    t["up1_out"][blk * C:(blk + 1) * C, :],
                )

    upool_cm.__exit__(None, None, None)

    # ------------------------------------------------------------------
    # P3: D2 16-NN + WeightNet aggregation -> warped
    # ------------------------------------------------------------------
    with tc.tile_pool(name="p3big", bufs=1) as p3big, \
         tc.tile_pool(name="p3s", bufs=1) as p3s:
        megaT = p3big.tile([P, 2, SW], F32, tag="megaT")
        nc.vector.memset(megaT[:], 0.0)
        for blk in range(4):
            s, off = blk // 2, (blk % 2) * NQ + (blk // 2)
            nc.sync.dma_start(
                megaT[:, s, off:off + NQ],
                t["mega_out"][blk * P:(blk + 1) * P, :].rearrange("c m -> c m"),
            )
        warp_loc = p3big.tile([6, NQ], F32, tag="warploc")
        for ti in range(NT):
            Vf, G = dmat_select(ti, t["k2aug"])
            T16 = p3s.tile([P, 16], F32, tag="d2t16")
            nc.vector.max(out=T16[:, 0:8], in_=Vf)
            Vr = p3s.tile([P, NCH * 8], F32, tag="d2vr")
            nc.vector.match_replace(out=Vr[:], in_to_replace=T16[:, 0:8],
                                    in_values=Vf, imm_value=NEG_BIG)
            nc.vector.max(out=T16[:, 8:16], in_=Vr[:])
            idx16 = mask_extract(p3s, Vf, G, T16[:], 16, "d2x")
            if DEBUG:
                nc.sync.dma_start(t["dbg_idx16"][:, ti * 16:(ti + 1) * 16], idx16[:])

            Gt = gather8k(megaT[:], idx16[:], p3big, "d2g")
            # token free layout: f = k*128 + q
            psb = ps_micro.tile([8, P], F32, tag="psu")
            nc.tensor.matmul(psb[:], wn1t_s[:], qrows_s[:, ti * P:(ti + 1) * P],
                             start=True, stop=True)
            bq = p3s.tile([8, P], F32, tag="bq")
            nc.vector.tensor_tensor(bq[:], wnb0_s[:].to_broadcast([8, P]), psb[:],
                                    ALU.subtract)
            Ag = p3s.tile([8, 2048], F32, tag="Ag")
            nc.sync.dma_start(Ag[:], Gt[96:104, :])
            h1 = p3big.tile([8, 2048], F32, tag="h1")
            nc.vector.tensor_tensor(
                h1[:].rearrange("c (q k) -> c q k", k=K),
                Ag[:].rearrange("c (q k) -> c q k", k=K),
                bq[:].unsqueeze(2).to_broadcast([8, P, K]), ALU.add,
            )
            nc.scalar.activation(h1[:], h1[:], AF.Relu)
            h2 = p3big.tile([8, 2048], F32, tag="h2")
            for ck in range(4):
                sl = slice(ck * 512, (ck + 1) * 512)
                ps = ps_micro.tile([8, 512], F32, tag="psu")
                nc.tensor.matmul(ps[:], wn2t_s[:], h1[:, sl], start=True, stop=True)
                nc.scalar.activation(h2[:, sl], ps[:], AF.Relu,
                                     bias=wnb1_s[:], scale=1.0)
            wgt = p3big.tile([W, 2048], F32, tag="wgt")
            for ck in range(4):
                sl = slice(ck * 512, (ck + 1) * 512)
                ps = ps_micro.tile([W, 512], F32, tag="psu")
                nc.tensor.matmul(ps[:], wn3t_s[:], h2[:, sl], start=True, stop=True)
                nc.scalar.activation(wgt[:, sl], ps[:], AF.Relu,
                                     bias=wnb2_s[:], scale=1.0)
            s6 = p3big.tile([6, 2048], F32, tag="s6")
            for ck in range(4):
                sl = slice(ck * 512, (ck + 1) * 512)
                ps = ps_micro.tile([96, 512], F32, tag="psu")
                nc.tensor.matmul(ps[:], wrep16_s[:], wgt[:, sl], start=True, stop=True)
                tchunk = p3s.tile([96, 512], F32, tag="tchunk")
                nc.vector.tensor_tensor(tchunk[:], ps[:], Gt[0:96, sl], ALU.mult)
                ps2 = ps_micro.tile([6, 512], F32, tag="psu")
                nc.tensor.matmul(ps2[:], selmat_s[:], tchunk[:], start=True, stop=True)
                nc.scalar.activation(s6[:, sl], ps2[:], AF.Copy)
            koff = p3s.tile([6, P], F32, tag="koff")
            nc.vector.tensor_reduce(
                koff[:], s6[:].rearrange("c (q k) -> c q k", k=K),
                axis=AXX, op=ALU.add,
            )
            kws = p3s.tile([W, P], F32, tag="kws")
            nc.vector.tensor_reduce(
                kws[:], wgt[:].rearrange("c (q k) -> c q k", k=K),
                axis=AXX, op=ALU.add,
            )
            psU = ps_micro.tile([96, P], F32, tag="psu")
            nc.tensor.matmul(psU[:], rxt_s[:], qrows_s[:, ti * P:(ti + 1) * P],
                             start=True, stop=True)
            psR = ps_micro.tile([96, P], F32, tag="psu")
            nc.tensor.matmul(psR[:], wrep16_s[:], kws[:], start=True, stop=True)
            kwrep = p3s.tile([96, P], F32, tag="kwrep")
            nc.scalar.activation(kwrep[:], psR[:], AF.Copy)
            umul = p3s.tile([96, P], F32, tag="umul")
            nc.vector.tensor_tensor(umul[:], kwrep[:], psU[:], ALU.mult)
            psC = ps_micro.tile([6, P], F32, tag="psu")
            nc.tensor.matmul(psC[:], selmat_s[:], umul[:], start=True, stop=True)
            off = p3s.tile([6, P], F32, tag="off")
            nc.vector.tensor_tensor(off[:], koff[:], psC[:], ALU.subtract)
            nc.vector.tensor_tensor(off[:], off[:], fcb_s[:].to_broadcast([6, P]),
                                    ALU.add)
            if DEBUG:
                nc.sync.dma_start(t["dbg_off"][:, ti * P:(ti + 1) * P], off[:])
            nc.vector.tensor_tensor(
                warp_loc[:, ti * P:(ti + 1) * P], off[:],
                qq6[:, ti * P:(ti + 1) * P], ALU.add,
            )
        nc.sync.dma_start(t["warp_in"][:], warp_loc[:])
        nc.gpsimd.collective_compute(
            "AllGather", ALU.bypass, replica_groups=GROUPS,
            ins=[t["warp_in"][:].opt()], outs=[t["warp_out"][:].opt()],
        )

    # ------------------------------------------------------------------
    # P4: warped aug tables
    # ------------------------------------------------------------------
    with tc.tile_pool(name="p4", bufs=1) as p4:
        for wi, dst in ((0, t["w1aug"]), (1, t["w2aug"])):
            for blk in range(4):
                csl = slice(blk * NQ, (blk + 1) * NQ)
                rowsc = p4.tile([3, NQ], F32, tag="rowsc")
                nc.sync.dma_start(
                    rowsc[:], t["warp_out"][blk * 6 + 3 * wi: blk * 6 + 3 * wi + 3, :])
                nc.sync.dma_start(dst[0:3, csl], rowsc[:])
                w2r = p4.tile([3, NQ], F32, tag="w2r")
                nc.vector.tensor_tensor(w2r[:], rowsc[:], rowsc[:], ALU.mult)
                nsq = p4.tile([1, NQ], F32, tag="wnsq")
                for ck in range(4):
                    sl = slice(ck * 512, (ck + 1) * 512)
                    psn = ps_micro.tile([1, 512], F32, tag="psu")
                    nc.tensor.matmul(psn[:], ones3_s[:], w2r[:, sl],
                                     start=True, stop=True)
                    nc.scalar.activation(nsq[:, sl], psn[:], AF.Copy)
                nc.sync.dma_start(dst[3:4, csl], nsq[:])

    # ------------------------------------------------------------------
    # P5: D3/D4 1-NN + final gather
    # ------------------------------------------------------------------
    with tc.tile_pool(name="p5s", bufs=2) as p5s:
        for ti in range(NT):
            idx12 = p5s.tile([P, 2], F32, tag="idx12")
            for wi, aug in ((0, t["w1aug"]), (1, t["w2aug"])):
                Vf, G = dmat_select(ti, aug)
                T8 = p5s.tile([P, 8], F32, tag="d34t8")
                nc.vector.max(out=T8[:], in_=Vf)
                idx1 = mask_extract(p5s, Vf, G, T8[:, 0:1], 1, "d34x")
                nc.vector.tensor_copy(idx12[:, wi:wi + 1], idx1[:])
            if DEBUG:
                nc.sync.dma_start(t["dbg_idx12"][:, ti * 2:(ti + 1) * 2], idx12[:])
            # per-group tables: groups 0..3 <- idx1, groups 4..7 <- idx2
            p1p = p5s.tile([P, 16], F32, tag="fgp1")
            nc.vector.tensor_copy(p1p[:], idx12[:, 0:1].to_broadcast([P, 16]))
            p2p = p5s.tile([P, 16], F32, tag="fgp2")
            nc.vector.tensor_copy(p2p[:], idx12[:, 1:2].to_broadcast([P, 16]))
            fg = gather8k(gtab[:], p1p[:], p5s, "fg", second=p2p[:])
            fgv = p5s.tile([P, P], F32, tag="fgv")
            nc.vector.tensor_copy(
                fgv[:], fg[:].rearrange("c (q jj) -> c q jj", jj=16)[:, :, 0])
            ps = ps_micro.tile([C, P], F32, tag="psu")
            nc.tensor.matmul(ps[:], sumsel_s[:], fgv[:], start=True, stop=True)
            nc.scalar.activation(outsb[:, ti * P:(ti + 1) * P], ps[:], AF.Copy)

    nc.sync.dma_start(t["out"][:], outsb[:])
    ctx.close()


# --------------------------------------------------------------------------
# host side
# --------------------------------------------------------------------------

_CACHE = {}


def _prep_inputs(inputs):
    xyz_1 = np.asarray(inputs["xyz_1"], np.float32)
    xyz_2 = np.asarray(inputs["xyz_2"], np.float32)
    feature_1 = np.asarray(inputs["feature_1"], np.float32)
    feature_2 = np.asarray(inputs["feature_2"], np.float32)
    conv1_w = np.asarray(inputs["conv1_w"], np.float32)
    fc_w = np.asarray(inputs["fc_w"], np.float32)
    fc3 = fc_w.reshape(6, OUT + 3, W)

    def bn2(x):
        return np.ascontiguousarray(np.asarray(x, np.float32).reshape(2, P).T)

    consts = dict(
        convwt=np.ascontiguousarray(conv1_w.T),
        bng=bn2(inputs["bn_gamma"]),
        bnb=bn2(inputs["bn_beta"]),
        bnm=bn2(inputs["bn_mean"]),
        bnv=bn2(inputs["bn_var"]),
        wn1t=np.ascontiguousarray(np.asarray(inputs["wn_w0"], np.float32).T),
        wn2t=np.ascontiguousarray(np.asarray(inputs["wn_w1"], np.float32).T),
        wn3t=np.ascontiguousarray(np.asarray(inputs["wn_w2"], np.float32).T),
        wnb0=np.asarray(inputs["wn_b0"], np.float32).reshape(8, 1),
        wnb1=np.asarray(inputs["wn_b1"], np.float32).reshape(8, 1),
        wnb2=np.asarray(inputs["wn_b2"], np.float32).reshape(W, 1),
        fcb=np.asarray(inputs["fc_b"], np.float32).reshape(6, 1),
    )
    rft = np.zeros((P, 192), np.float32)
    rfull = fc3[:, 3:, :]                       # [6, 256, 16]
    for h in range(2):
        blk = rfull[:, h * P:(h + 1) * P, :]    # [6, 128, 16]
        rft[:, h * 96:(h + 1) * 96] = blk.transpose(1, 0, 2).reshape(P, 96)
    consts["rft"] = rft
    consts["rxt"] = np.ascontiguousarray(
        fc3[:, 0:3, :].transpose(1, 0, 2).reshape(3, 96))
    selmat = np.zeros((96, 6), np.float32)
    for o in range(6):
        selmat[o * 16:(o + 1) * 16, o] = 1.0
    consts["selmat"] = selmat
    wrep = np.zeros((W, 96), np.float32)
    for o in range(6):
        wrep[:, o * 16:(o + 1) * 16] = np.eye(W, dtype=np.float32)
    consts["wrep16"] = wrep
    sumsel = np.zeros((P, C), np.float32)
    for c in range(C):
        sumsel[c, c] = 1.0
        sumsel[c + C, c] = 1.0
    consts["sumsel"] = sumsel
    consts["ident"] = np.eye(P, dtype=np.float32)
    gb = (float(CH) * (np.arange(NCH * 8) // 8)).astype(np.float32)
    consts["gbase"] = np.broadcast_to(gb, (P, NCH * 8)).copy()

    def pt128(rows):
        return np.ascontiguousarray(rows.T.reshape(64, P, 3).transpose(1, 0, 2))

    in_maps = []
    for core in range(8):
        b, r = core // 4, core % 4
        q0 = r * NQ
        x1, x2 = xyz_1[b], xyz_2[b]
        m = dict(consts)
        m.update(
            k1rows=np.ascontiguousarray(x1),
            k2rows=np.ascontiguousarray(x2),
            k1pt=pt128(x1),
            k2pt=pt128(x2),
            qrows=np.ascontiguousarray(x2[:, q0:q0 + NQ]),
            qpt=np.ascontiguousarray(
                x2[:, q0:q0 + NQ].T.reshape(NT, P, 3).transpose(1, 0, 2)),
            f1rows=np.ascontiguousarray(feature_1[b]),
            f2loc=np.ascontiguousarray(feature_2[b][:, q0:q0 + NQ]),
            f2rows=np.ascontiguousarray(feature_2[b]),
        )
        in_maps.append(m)
    return in_maps


def kernel(**inputs):
    if "nc" not in _CACHE:
        _CACHE["nc"] = build_program()
    nc = _CACHE["nc"]
    in_maps = _prep_inputs(inputs)
    res = run_bass_kernel_spmd(nc, in_maps, core_ids=list(range(8)))
    _CACHE["last_res"] = res
    out = np.zeros((B, N, C), np.float32)
    for core in range(8):
        b, r = core // 4, core % 4
        out[b, r * NQ:(r + 1) * NQ, :] = res.results[core]["out"].T
    if DEBUG:
        _CACHE["results"] = res.results
    return out



# revision 56
# speedup vs baseline: 1.2602x; 1.2602x over previous
"""Trainium2 Bass kernel for nn_Aligned_Feature_Aggregation.

Pipeline (B=2, N=8192, C=64, OUT=256, W=16, K=16):
  up1 = 3-NN inverse-distance interp of feature_1 (at xyz_1) onto xyz_2
  nf  = LeakyReLU(BN(conv1([up1; feature_2])))
  idx = 16-NN self KNN of xyz_2; WeightNet aggregation -> offsets [6, N]
  warped_{1,2} = xyz_2 + offsets; two 1-NN lookups against warped sets
  out = up1^T[idx1] + feature_2^T[idx2]   -> [B, N, 64]

Sharding: 8 cores = 2 batches x 4 query-quarters (2048 queries each).
Keys replicated per batch; AllGather (groups of 4) shares nf-derived fold
tables, up1 and the warped point sets.

Distance matrices are computed as Dt = 2*q.k - |k|^2 on the PE (fp32,
4-dim lifted contraction); bigger = nearer. Top-k per 1024-key chunk via
DVE max8 + max_index, then a mask-based merge (ties -> lowest index).
The [B,N,K,259]x[B,N,K,16] aggregation is folded through fc_w so only a
96-channel per-key table is gathered (R_all = fc3 . [k_xyz; nf]).
"""

import os
import numpy as np

import concourse.bass as bass
import concourse.mybir as mybir
import concourse.tile as tile_mod
from concourse.bass_utils import run_bass_kernel_spmd
from concourse.vector_clock import ScopedClock

F32 = mybir.dt.float32
BF16 = mybir.dt.bfloat16
U16 = mybir.dt.uint16
I16 = mybir.dt.int16
ALU = mybir.AluOpType
AF = mybir.ActivationFunctionType
AXX = mybir.AxisListType.X

B, N, C = 2, 8192, 64
OUT, W, K = 256, 16, 16
NQ = 2048            # queries per core
NT = 16              # query tiles of 128 per core
NCH = 8              # key chunks per tile
CH = 1024            # key chunk size
P = 128
BN_EPS = 1e-5
LEAKY = 0.1
NEG_BIG = -3.0e38
GROUPS = [[0, 1, 2, 3], [4, 5, 6, 7]]

DEBUG = bool(int(os.environ.get("BASS_KERNEL_DEBUG", "0")))


def _patch_tile_drain():
    """walrus in this env rejects >1 sem wait on the final SP drain; split."""
    if getattr(tile_mod.TileContext, "_drain_split_patched", False):
        return

    def _drain_and_barrier(self, tick_clock, wait_clock):
        nc = self.nc
        drain_inst = nc.sync.drain()
        wait_clock.add_sem_waits(
            drain_inst.ins, ScopedClock({None: tick_clock.global_clock})
        )
        si = drain_inst.ins.sync_info
        if si is not None and si.on_wait is not None and len(si.on_wait) > 1:
            waits = list(si.on_wait)
            si.on_wait = waits[:1]
            for w in waits[1:]:
                d2 = nc.sync.drain()
                d2.ins.sync_info = mybir.SyncInfo(on_wait=[w], on_update=[])
        nc.all_engine_barrier()
        assert self.sems is not None
        popped = nc._tile_sem_poison_stack.pop()
        assert popped is self._sem_poison
        nc.clear_and_free_semaphores(list(self.sems.allocated().values()))
        nc.all_engine_barrier()

    tile_mod.TileContext._drain_and_barrier = _drain_and_barrier
    tile_mod.TileContext._drain_split_patched = True


def build_program():
    _patch_tile_drain()
    nc = bass.Bass("TRN2", target_bir_lowering=False, debug=False)

    def din(name, shape):
        return nc.dram_tensor(name, shape, F32, kind="ExternalInput").ap()

    t = {}
    t["k1rows"] = din("k1rows", [3, N])
    t["k2rows"] = din("k2rows", [3, N])
    t["k1pt"] = din("k1pt", [P, 64, 3])
    t["k2pt"] = din("k2pt", [P, 64, 3])
    t["qrows"] = din("qrows", [3, NQ])
    t["qpt"] = din("qpt", [P, NT, 3])
    t["f1rows"] = din("f1rows", [C, N])
    t["f2loc"] = din("f2loc", [C, NQ])
    t["f2rows"] = din("f2rows", [C, N])
    t["convwt"] = din("convwt", [P, OUT])
    for nm in ("bng", "bnb", "bnm", "bnv"):
        t[nm] = din(nm, [P, 2])
    t["wn1t"] = din("wn1t", [3, 8])
    t["wn2t"] = din("wn2t", [8, 8])
    t["wn3t"] = din("wn3t", [8, W])
    t["wnb0"] = din("wnb0", [8, 1])
    t["wnb1"] = din("wnb1", [8, 1])
    t["wnb2"] = din("wnb2", [W, 1])
    t["rft"] = din("rft", [P, 192])
    t["rxt"] = din("rxt", [3, 96])
    t["repmat"] = din("repmat", [16, P])
    t["fcb"] = din("fcb", [6, 1])
    t["selmat"] = din("selmat", [96, 6])
    t["wrep16"] = din("wrep16", [W, 96])
    t["sumsel"] = din("sumsel", [P, C])
    t["ident"] = din("ident", [P, P])
    t["gbase"] = din("gbase", [P, NCH * 8])

    t["out"] = nc.dram_tensor("out", [C, NQ], F32, kind="ExternalOutput").ap()
    if DEBUG:
        for nm, shp in [
            ("dbg_idx3", [P, NT * 3]),
            ("dbg_up1t", [C, NQ]),
            ("dbg_nf", [P, 2 * NQ]),
            ("dbg_idx16", [P, NT * 16]),
            ("dbg_off", [6, NQ]),
            ("dbg_idx12", [P, NT * 2]),
        ]:
            t[nm] = nc.dram_tensor(nm, shp, F32, kind="ExternalOutput").ap()

    t["augrow"] = nc.dram_tensor("augrow_d", [4, N], BF16).ap()
    t["mega_in"] = nc.dram_tensor("mega_in", [P, NQ], F32).ap()
    t["mega_out"] = nc.dram_tensor("mega_out", [4 * P, NQ], F32).ap()
    t["up1_in"] = nc.dram_tensor("up1_in", [C, NQ], F32).ap()
    t["up1_out"] = nc.dram_tensor("up1_out", [4 * C, NQ], F32).ap()
    t["warp_in"] = nc.dram_tensor("warp_in", [6, NQ], F32).ap()
    t["warp_out"] = nc.dram_tensor("warp_out", [24, NQ], F32).ap()

    with tile_mod.TileContext(nc) as tc:
        _build(nc, tc, t)
    _split_excess_waits(nc)
    return nc


def _split_excess_waits(nc, limit=1):
    """walrus rejects >2 sync waits per instruction: hoist extras onto NoOps."""
    for bbh in nc.bb_map.values():
        inner = bbh.bb
        insts = inner.instructions
        out = []
        changed = False
        for inst in insts:
            si = inst.sync_info
            waits = list(si.on_wait) if si is not None and si.on_wait else []
            if len(waits) > limit:
                excess, keep = waits[:-limit], waits[-limit:]
                for j in range(0, len(excess), limit):
                    nop = mybir.InstNoOp(
                        name=f"{inst.name}-ws{j}", ins=[], outs=[])
                    nop.engine = inst.engine
                    nop.sync_info = mybir.SyncInfo(
                        on_wait=excess[j:j + limit], on_update=[])
                    out.append(nop)
                si.on_wait = keep
                changed = True
            out.append(inst)
        if changed:
            inner.instructions = out


def _build(nc, tc, t):
    import contextlib
    ctx = contextlib.ExitStack()


    # ------------------------------------------------------------------
    # persistent SBUF
    # ------------------------------------------------------------------
    persist = ctx.enter_context(tc.tile_pool(name="persist", bufs=1))
    gtab = persist.tile([P, 2, 4112], F32, tag="gtab")  # rows 0:64 up1(AG), 64:128 f2
    # Distance matmuls run in bf16 hi/lo split form: 11 contraction rows
    # computing q_hi.k_hi + q_lo.k_hi + q_hi.k_lo (error ~2^-16).
    # Query aug [11 rows], replicated at base partitions 0 and 32 to match
    # the two key-table slots (matmul requires equal base partitions):
    # rows 0:3 2q_hi, 3 -1, 4:7 2q_lo, 7 -1, 8:11 2q_hi.
    qaugr = persist.tile([43, NQ], BF16, tag="qaugr")
    # Packed key-aug tables, SBUF resident (PE base partition must be
    # 0/32/64): rows r0+0:3 k_hi, r0+3 s_hi, r0+4:7 k_hi, r0+7 s_lo,
    # r0+8:11 k_lo.  r0=0: k1 (later warped1), r0=32: k2 (later warped2).
    augsb = persist.tile([43, N], BF16, tag="augsb")
    outsb = persist.tile([C, NQ], F32, tag="outsb")

    consts = ctx.enter_context(tc.tile_pool(name="consts", bufs=1))

    def load_const(name, shape):
        s = consts.tile(shape, F32, tag="c_" + name)
        nc.sync.dma_start(s[:], t[name][:])
        return s

    convwt_lo = consts.tile([C, OUT], F32, tag="c_convlo")
    nc.sync.dma_start(convwt_lo[:], t["convwt"][0:C, :])
    convwt_hi = consts.tile([C, OUT], F32, tag="c_convhi")
    nc.sync.dma_start(convwt_hi[:], t["convwt"][C:P, :])
    wn1t_s = load_const("wn1t", [3, 8])
    wn2t_s = load_const("wn2t", [8, 8])
    wn3t_s = load_const("wn3t", [8, W])
    wnb0_s = load_const("wnb0", [8, 1])
    wnb1_s = load_const("wnb1", [8, 1])
    wnb2_s = load_const("wnb2", [W, 1])
    rft_s = load_const("rft", [P, 192])
    rxt_s = load_const("rxt", [3, 96])
    repmat_s = load_const("repmat", [16, P])
    fcb_s = load_const("fcb", [6, 1])
    selmat_s = load_const("selmat", [96, 6])
    wrep16_s = load_const("wrep16", [W, 96])
    sumsel_s = load_const("sumsel", [P, C])
    ident_s = load_const("ident", [P, P])
    gbase_s = load_const("gbase", [P, NCH * 8])
    ones3_s = consts.tile([3, 1], F32, tag="c_ones3")
    nc.vector.memset(ones3_s[:], 1.0)
    ones64_s = consts.tile([1, C], F32, tag="c_ones64")
    nc.vector.memset(ones64_s[:], 1.0)
    qrows_s = load_const("qrows", [3, NQ])
    qpt_s = load_const("qpt", [P, NT, 3])

    # BN scale/bias: scale = g/sqrt(v+eps), bias = b - m*scale
    bn = consts.tile([P, 2, 4], F32, tag="bn")
    nc.sync.dma_start(bn[:, :, 0], t["bng"][:])
    nc.sync.dma_start(bn[:, :, 1], t["bnb"][:])
    nc.sync.dma_start(bn[:, :, 2], t["bnm"][:])
    nc.sync.dma_start(bn[:, :, 3], t["bnv"][:])
    bnsc = consts.tile([P, 2, 2], F32, tag="bnsc")
    tmpbn = consts.tile([P, 2], F32, tag="tmpbn")
    nc.vector.tensor_scalar_add(tmpbn[:], bn[:, :, 3], float(BN_EPS))
    nc.scalar.activation(tmpbn[:], tmpbn[:], AF.Sqrt)
    nc.vector.reciprocal(tmpbn[:], tmpbn[:])
    nc.vector.tensor_tensor(bnsc[:, :, 0], bn[:, :, 0], tmpbn[:], ALU.mult)
    nc.vector.tensor_tensor(tmpbn[:], bn[:, :, 2], bnsc[:, :, 0], ALU.mult)
    nc.vector.tensor_tensor(bnsc[:, :, 1], bn[:, :, 1], tmpbn[:], ALU.subtract)

    # qaug = [2*q; -1] -> bf16 hi/lo split, staged through a scoped pool
    with tc.tile_pool(name="qtmp", bufs=1) as qtmp:
        qaug = qtmp.tile([4, NQ], F32, tag="qaug")
        nc.vector.memset(qaug[:], -1.0)
        nc.scalar.mul(qaug[0:3, :], qrows_s[:], 2.0)
        qhi = qtmp.tile([4, NQ], BF16, tag="qhi")
        nc.scalar.activation(qhi[:], qaug[:], AF.Copy)
        qlo = qtmp.tile([4, NQ], BF16, tag="qlo")
        nc.vector.tensor_tensor(qlo[:], qaug[:], qhi[:], ALU.subtract)
        for b in (0, 32):
            nc.sync.dma_start(qaugr[b:b + 4, :], qhi[:])
            nc.sync.dma_start(qaugr[b + 4:b + 7, :], qlo[0:3, :])
            nc.sync.dma_start(qaugr[b + 7:b + 8, :], qhi[3:4, :])
            nc.sync.dma_start(qaugr[b + 8:b + 11, :], qhi[0:3, :])
    # qq6 = [q; q] for warped = offset + xyz2
    qq6 = persist.tile([6, NQ], F32, tag="qq6")
    nc.sync.dma_start(qq6[0:3, :], t["qrows"][:])
    nc.sync.dma_start(qq6[3:6, :], t["qrows"][:])

    # long-lived scratch pools
    mm_pool = ctx.enter_context(tc.tile_pool(name="mmp", bufs=3, space="PSUM"))
    ps_micro = ctx.enter_context(tc.tile_pool(name="psmicro", bufs=2, space="PSUM"))
    sel_pool = ctx.enter_context(tc.tile_pool(name="selp", bufs=3))
    tabp = ctx.enter_context(tc.tile_pool(name="tabp", bufs=3))

    # ------------------------------------------------------------------
    # helpers
    # ------------------------------------------------------------------
    def build_aug(pool, rows_src, pt_src_dram, r0, stage_hi, stage_lo):
        """bf16 hi/lo aug block at augsb[r0:r0+11] (see layout above)."""
        rows_f = pool.tile([3, N], F32, tag="augrows")
        nc.sync.dma_start(rows_f[:], rows_src)
        rhi = pool.tile([3, N], BF16, tag="aughi")
        nc.scalar.activation(rhi[:], rows_f[:], AF.Copy)
        rlo = pool.tile([3, N], BF16, tag="auglo")
        nc.vector.tensor_tensor(rlo[:], rows_f[:], rhi[:], ALU.subtract)
        nc.sync.dma_start(augsb[r0:r0 + 3, :], rhi[:])
        nc.sync.dma_start(augsb[r0 + 4:r0 + 7, :], rhi[:])
        nc.sync.dma_start(augsb[r0 + 8:r0 + 11, :], rlo[:])
        ptt = pool.tile([P, 64, 3], F32, tag="augpt")
        nc.scalar.dma_start(ptt[:], pt_src_dram)
        sq = pool.tile([P, 64, 3], F32, tag="augsq")
        nc.vector.tensor_tensor(sq[:], ptt[:], ptt[:], ALU.mult)
        nsq = pool.tile([P, 64], F32, tag="augn")
        nc.vector.tensor_reduce(nsq[:], sq[:], axis=AXX, op=ALU.add)
        shi = pool.tile([P, 64], BF16, tag="augshi")
        nc.scalar.activation(shi[:], nsq[:], AF.Copy)
        slo = pool.tile([P, 64], BF16, tag="augslo")
        nc.vector.tensor_tensor(slo[:], nsq[:], shi[:], ALU.subtract)
        # s row col (g*128+p) <- s[p, g]; the partition-crossing scatter
        # needs HWDGE + a DRAM bounce (SBUF->SBUF can't balance the AP).
        for src, stage, rr in ((shi, stage_hi, 3), (slo, stage_lo, 7)):
            nc.sync.dma_start(
                stage.rearrange("one (g p) -> one p g", p=P), src[:])
            nc.sync.dma_start(augsb[r0 + rr:r0 + rr + 1, :], stage)

    def dmat_select(ti, r0):
        """f32r distance matmuls + per-chunk top8.  Returns (V, G) [P, 64]."""
        V = sel_pool.tile([P, NCH, 8], F32, tag="selV")
        Gu = sel_pool.tile([P, NCH, 8], U16, tag="selGu")
        lhs = qaugr[r0:r0 + 11, ti * P:(ti + 1) * P]
        for cki in range(NCH):
            ps = mm_pool.tile([P, CH], F32, tag="dmat")
            rhs = augsb[r0:r0 + 11, cki * CH:(cki + 1) * CH]
            for h in range(2):
                nc.tensor.matmul(
                    ps[:, h * 512:(h + 1) * 512], lhs,
                    rhs[:, h * 512:(h + 1) * 512], start=True, stop=True,
                )
            nc.vector.max(out=V[:, cki, :], in_=ps[:])
            nc.vector.max_index(out=Gu[:, cki, :], in_max=V[:, cki, :], in_values=ps[:])
        Vf = V[:].rearrange("p a b -> p (a b)")
        G = sel_pool.tile([P, NCH * 8], F32, tag="selGf")
        nc.vector.tensor_tensor(
            G[:], Gu[:].rearrange("p a b -> p (a b)"), gbase_s[:], ALU.add)
        return Vf, G

    def mask_extract(pool, Vf, G, ranks_ap, nk, tag):
        """idx[p, j] = G[p, pos(Vf == ranks[j])]; ties -> min index."""
        ncand = NCH * 8
        m = pool.tile([P, nk, ncand], mybir.dt.uint8, tag=tag + "m")
        nc.vector.tensor_tensor(
            m[:], Vf.unsqueeze(1).to_broadcast([P, nk, ncand]),
            ranks_ap.unsqueeze(2).to_broadcast([P, nk, ncand]), ALU.is_equal,
        )
        sel = pool.tile([P, nk, ncand], F32, tag=tag + "s")
        nc.vector.memset(sel[:], 65535.0)
        nc.vector.copy_predicated(
            sel[:], m[:], G[:].unsqueeze(1).to_broadcast([P, nk, ncand])
        )
        idx = pool.tile([P, nk], F32, tag=tag + "i")
        nc.vector.tensor_reduce(idx[:], sel[:], axis=AXX, op=ALU.min)
        return idx

    def transpose_pe(src_ap, m, tag):
        """[128, m<=128] -> PSUM [m, 128]"""
        ps = ps_micro.tile([m, P], F32, tag="psu")
        nc.tensor.matmul(ps[:], src_ap, ident_s[:], is_transpose=True)
        return ps

    def build_tab16(idxf16_ap, tag, second=None):
        """idxf16_ap [128, 16] f32 -> int16 ap_gather table [128, 128].

        Token i = q*16 + jj: unwrapped[i] = idxf16[q, jj] for every
        16-partition group. If `second` is given, groups 4..7 use it
        instead (per-group tables). Replication across the 8 groups is
        done on the PE via the block-replicate const `repmat`.
        """
        srcs = []
        for s_ap, stag in ((idxf16_ap, "a"), (second, "b")):
            if s_ap is None:
                srcs.append(None)
                continue
            tp = transpose_pe(s_ap, 16, tag + stag)
            tps = tabp.tile([16, P], F32, tag="ttps" + stag)
            nc.scalar.activation(tps[:], tp[:], AF.Copy)
            srcs.append(tps)
        tab = tabp.tile([P, P], U16, tag="ttabi")
        if second is None:
            tabP = ps_micro.tile([P, P], F32, tag="psu")
            nc.tensor.matmul(tabP[:], repmat_s[:], srcs[0][:],
                             start=True, stop=True)
            nc.vector.tensor_copy(tab[:], tabP[:])
        else:
            tabA = ps_micro.tile([64, P], F32, tag="psu")
            nc.tensor.matmul(tabA[:], repmat_s[:, 0:64], srcs[0][:],
                             start=True, stop=True)
            tabB = ps_micro.tile([64, P], F32, tag="psu")
            nc.tensor.matmul(tabB[:], repmat_s[:, 0:64], srcs[1][:],
                             start=True, stop=True)
            nc.vector.tensor_copy(tab[0:64, :], tabA[:])
            nc.vector.tensor_copy(tab[64:P, :], tabB[:])
        return tab

    SW = 4112  # split-slice width (sentinel zero col at 4096 / 0)

    def gather8k(data2_ap, idx16f_ap, gpool, gtag, second=None):
        """Gather [128ch, 2048tok] from a [128, 2, 4112] split table.

        slice0 cols 0:4096 = keys 0:4096's data? actually keys 0:4095 at
        cols 0:4095, col 4096 = zero sentinel; slice1 col 0 = zero sentinel,
        cols 1:4096 = keys 4096:8191. idx16f in [0, 8192).
        """
        outs = []
        for which, idxsrc in (("a", idx16f_ap), ("b", second)):
            if idxsrc is None:
                idxsrc = idx16f_ap
            ia = tabp.tile([P, 16], F32, tag="gsa" + which)
            nc.vector.tensor_scalar_min(ia[:], idxsrc, 4096.0)
            ib = tabp.tile([P, 16], F32, tag="gsb" + which)
            nc.vector.tensor_scalar(
                ib[:], idxsrc, 4095.0, scalar2=0.0,
                op0=ALU.subtract, op1=ALU.max)
            outs.append((ia, ib))
        if second is None:
            tabA = build_tab16(outs[0][0][:], gtag + "A")
            tabB = build_tab16(outs[0][1][:], gtag + "B")
        else:
            tabA = build_tab16(outs[0][0][:], gtag + "A", second=outs[1][0][:])
            tabB = build_tab16(outs[0][1][:], gtag + "B", second=outs[1][1][:])
        gA = gpool.tile([P, 2048], F32, tag=gtag + "gA")
        gB = gpool.tile([P, 2048], F32, tag=gtag + "gB")
        for h in range(2):
            hsl = slice(h * 1024, (h + 1) * 1024)
            tsl = slice(h * 64, (h + 1) * 64)
            nc.gpsimd.indirect_copy(gA[:, hsl], data2_ap[:, 0, :], tabA[:, tsl], True)
            nc.gpsimd.indirect_copy(gB[:, hsl], data2_ap[:, 1, :], tabB[:, tsl], True)
        nc.vector.tensor_tensor(gA[:], gA[:], gB[:], ALU.add)
        return gA

    # ------------------------------------------------------------------
    # P0: key aug tables
    # ------------------------------------------------------------------
    with tc.tile_pool(name="p0", bufs=1) as p0:
        build_aug(p0, t["k1rows"][:], t["k1pt"][:], 0,
                  t["augrow"][0:1, :], t["augrow"][1:2, :])
        build_aug(p0, t["k2rows"][:], t["k2pt"][:], 32,
                  t["augrow"][2:3, :], t["augrow"][3:4, :])
    # f2 -> gtab rows 64:128 (split layout, sentinel cols zero)
    nc.vector.memset(gtab[:], 0.0)
    nc.sync.dma_start(gtab[C:P, 0, 0:4096], t["f2rows"][:, 0:4096])
    nc.sync.dma_start(gtab[C:P, 1, 1:4097], t["f2rows"][:, 4096:N])

    # ------------------------------------------------------------------
    # P1: D1 3-NN + upsample -> up1T
    # ------------------------------------------------------------------
    upool_cm = tc.tile_pool(name="upool", bufs=1)
    upool = upool_cm.__enter__()
    up1T = upool.tile([C, NQ], F32, tag="up1T")
    with tc.tile_pool(name="p1big", bufs=1) as p1big, \
         tc.tile_pool(name="p1w", bufs=1) as p1w, \
         tc.tile_pool(name="p1s", bufs=2) as p1s:
        # split gather table [128, 2, 4112]: rows 0:3 xyz1, 64:128 feat1
        p1sb = p1big.tile([P, 2, SW], F32, tag="p1sb")
        nc.vector.memset(p1sb[:], 0.0)
        nc.sync.dma_start(p1sb[0:3, 0, 0:4096], t["k1rows"][:, 0:4096])
        nc.sync.dma_start(p1sb[0:3, 1, 1:4097], t["k1rows"][:, 4096:N])
        nc.sync.dma_start(p1sb[C:P, 0, 0:4096], t["f1rows"][:, 0:4096])
        nc.sync.dma_start(p1sb[C:P, 1, 1:4097], t["f1rows"][:, 4096:N])
        for ti in range(NT):
            Vf, G = dmat_select(ti, 0)
            T8 = p1s.tile([P, 8], F32, tag="d1t8")
            nc.vector.max(out=T8[:], in_=Vf)
            idx3 = mask_extract(p1s, Vf, G, T8[:, 0:3], 3, "d1x")
            if DEBUG:
                nc.sync.dma_start(t["dbg_idx3"][:, ti * 3:(ti + 1) * 3], idx3[:])
            idx16p = p1s.tile([P, 16], F32, tag="d1pad")
            nc.vector.tensor_copy(idx16p[:, 0:3], idx3[:])
            nc.vector.tensor_copy(
                idx16p[:, 3:16], idx3[:, 0:1].to_broadcast([P, 13]))
            gout = gather8k(p1sb[:], idx16p[:], p1w, "p1g")
            # token f = q*16 + jj (jj<3 used). rows 0:3 xyz1, 3:67 feat1
            gx = p1s.tile([3, P, 3], F32, tag="upg")
            nc.vector.tensor_tensor(
                gx[:], gout[0:3, :].rearrange("c (q jj) -> c q jj", jj=16)[:, :, 0:3],
                qrows_s[:, ti * P:(ti + 1) * P].unsqueeze(2).to_broadcast([3, P, 3]),
                ALU.subtract,
            )
            nc.vector.tensor_tensor(gx[:], gx[:], gx[:], ALU.mult)
            gxs = p1s.tile([3, P * 3], F32, tag="upgs")
            nc.vector.tensor_copy(gxs[:], gx[:].rearrange("c q jj -> c (q jj)"))
            psd = ps_micro.tile([1, P * 3], F32, tag="psu")
            nc.tensor.matmul(psd[:], ones3_s[:], gxs[:], start=True, stop=True)
            dist = p1s.tile([1, P, 3], F32, tag="updist")
            nc.scalar.activation(
                dist[:].rearrange("one q jj -> one (q jj)"), psd[:], AF.Sqrt)
            nc.vector.tensor_scalar_max(dist[:], dist[:], 1e-10)
            w3 = p1s.tile([1, P, 3], F32, tag="upw")
            nc.vector.reciprocal(w3[:], dist[:])
            wsum = p1s.tile([1, P], F32, tag="upws")
            nc.vector.tensor_reduce(wsum[:], w3[:], axis=AXX, op=ALU.add)
            nc.vector.reciprocal(wsum[:], wsum[:])
            nc.vector.tensor_tensor(
                w3[:], w3[:], wsum[:].unsqueeze(2).to_broadcast([1, P, 3]), ALU.mult)
            # replicate wn to 64 partitions via PE, padded to 16 jj (zeros)
            wn16 = p1w.tile([1, P, 16], F32, tag="wn16")
            nc.vector.memset(wn16[:], 0.0)
            nc.vector.tensor_copy(wn16[:, :, 0:3], w3[:])
            wrep = p1w.tile([P, 2048], F32, tag="uwrep")
            for ck in range(4):
                psx = ps_micro.tile([C, 512], F32, tag="psu")
                nc.tensor.matmul(
                    psx[:], ones64_s[:],
                    wn16[:].rearrange("one q jj -> one (q jj)")[:, ck * 512:(ck + 1) * 512],
                    start=True, stop=True,
                )
                nc.scalar.activation(wrep[C:P, ck * 512:(ck + 1) * 512], psx[:], AF.Copy)
            wf = p1w.tile([P, 2048], F32, tag="upwf")
            nc.vector.tensor_tensor(wf[C:P, :], gout[C:P, :], wrep[C:P, :], ALU.mult)
            nc.vector.tensor_reduce(
                up1T[:, ti * P:(ti + 1) * P],
                wf[C:P, :].rearrange("c (q jj) -> c q jj", jj=16),
                axis=AXX, op=ALU.add,
            )

    if DEBUG:
        nc.sync.dma_start(t["dbg_up1t"][:], up1T[:])

    # ------------------------------------------------------------------
    # P2: conv/BN/LeakyReLU -> nf; fold tables; AllGathers
    # ------------------------------------------------------------------
    if True:
        with tc.tile_pool(name="p2", bufs=1) as p2:
            f2loc_s = p2.tile([C, NQ], F32, tag="f2loc")
            nc.sync.dma_start(f2loc_s[:], t["f2loc"][:])
            nfsb = p2.tile([P, 2, NQ], F32, tag="nfsb")
            for h in range(2):
                for ck in range(4):
                    sl = slice(ck * 512, (ck + 1) * 512)
                    ps = ps_micro.tile([P, 512], F32, tag="psu")
                    nc.tensor.matmul(ps[:], convwt_lo[:, h * P:(h + 1) * P],
                                     up1T[:, sl], start=True, stop=False)
                    nc.tensor.matmul(ps[:], convwt_hi[:, h * P:(h + 1) * P],
                                     f2loc_s[:, sl], start=False, stop=True)
                    nc.scalar.activation(
                        nfsb[:, h, sl], ps[:], AF.Copy,
                        bias=0.0, scale=bnsc[:, h, 0:1],
                    )
                    # Copy ignores AP bias; add bias then LeakyReLU = max(x, 0.1x)
                    nc.vector.tensor_tensor(
                        nfsb[:, h, sl], nfsb[:, h, sl],
                        bnsc[:, h, 1:2].to_broadcast([P, 512]), ALU.add)
                    nc.vector.scalar_tensor_tensor(
                        nfsb[:, h, sl], nfsb[:, h, sl], LEAKY, nfsb[:, h, sl],
                        op0=ALU.mult, op1=ALU.max)
            if DEBUG:
                nc.sync.dma_start(
                    t["dbg_nf"][:], nfsb[:].rearrange("p a b -> p (a b)"))

            mega_loc = p2.tile([P, NQ], F32, tag="megaloc")
            nc.vector.memset(mega_loc[:], 0.0)
            for ck in range(4):
                sl = slice(ck * 512, (ck + 1) * 512)
                ps = ps_micro.tile([96, 512], F32, tag="psu")
                nc.tensor.matmul(ps[:], rft_s[:, 0:96], nfsb[:, 0, sl],
                                 start=True, stop=False)
                nc.tensor.matmul(ps[:], rft_s[:, 96:192], nfsb[:, 1, sl],
                                 start=False, stop=False)
                nc.tensor.matmul(ps[:], rxt_s[:], qrows_s[:, sl],
                                 start=False, stop=True)
                nc.scalar.activation(mega_loc[0:96, sl], ps[:], AF.Copy)
                ps2 = ps_micro.tile([8, 512], F32, tag="psu")
                nc.tensor.matmul(ps2[:], wn1t_s[:], qrows_s[:, sl],
                                 start=True, stop=True)
                nc.scalar.activation(mega_loc[96:104, sl], ps2[:], AF.Copy)

            nc.sync.dma_start(t["mega_in"][:], mega_loc[:])
            nc.gpsimd.collective_compute(
                "AllGather", ALU.bypass, replica_groups=GROUPS,
                ins=[t["mega_in"][:].opt()], outs=[t["mega_out"][:].opt()],
            )
            nc.sync.dma_start(t["up1_in"][:], up1T[:])
            nc.gpsimd.collective_compute(
                "AllGather", ALU.bypass, replica_groups=GROUPS,
                ins=[t["up1_in"][:].opt()], outs=[t["up1_out"][:].opt()],
            )
            for blk in range(4):
                s, off = blk // 2, (blk % 2) * NQ + (blk // 2)
                nc.sync.dma_start(
                    gtab[0:C, s, off:off + NQ],
                    t["up1_out"][blk * C:(blk + 1) * C, :],
                )

    upool_cm.__exit__(None, None, None)

    # ------------------------------------------------------------------
    # P3: D2 16-NN + WeightNet aggregation -> warped
    # ------------------------------------------------------------------
    with tc.tile_pool(name="p3big", bufs=1) as p3big, \
         tc.tile_pool(name="p3s", bufs=1) as p3s:
        megaT = p3big.tile([P, 2, SW], F32, tag="megaT")
        nc.vector.memset(megaT[:], 0.0)
        for blk in range(4):
            s, off = blk // 2, (blk % 2) * NQ + (blk // 2)
            nc.sync.dma_start(
                megaT[:, s, off:off + NQ],
                t["mega_out"][blk * P:(blk + 1) * P, :].rearrange("c m -> c m"),
            )
        warp_loc = p3big.tile([6, NQ], F32, tag="warploc")
        for ti in range(NT):
            Vf, G = dmat_select(ti, 32)
            T16 = p3s.tile([P, 16], F32, tag="d2t16")
            nc.vector.max(out=T16[:, 0:8], in_=Vf)
            Vr = p3s.tile([P, NCH * 8], F32, tag="d2vr")
            nc.vector.match_replace(out=Vr[:], in_to_replace=T16[:, 0:8],
                                    in_values=Vf, imm_value=NEG_BIG)
            nc.vector.max(out=T16[:, 8:16], in_=Vr[:])
            idx16 = mask_extract(p3s, Vf, G, T16[:], 16, "d2x")
            if DEBUG:
                nc.sync.dma_start(t["dbg_idx16"][:, ti * 16:(ti + 1) * 16], idx16[:])

            Gt = gather8k(megaT[:], idx16[:], p3big, "d2g")
            # token free layout: f = k*128 + q
            psb = ps_micro.tile([8, P], F32, tag="psu")
            nc.tensor.matmul(psb[:], wn1t_s[:], qrows_s[:, ti * P:(ti + 1) * P],
                             start=True, stop=True)
            bq = p3s.tile([8, P], F32, tag="bq")
            nc.vector.tensor_tensor(bq[:], wnb0_s[:].to_broadcast([8, P]), psb[:],
                                    ALU.subtract)
            h1 = p3big.tile([8, 2048], F32, tag="h1")
            nc.sync.dma_start(h1[:], Gt[96:104, :])
            nc.vector.tensor_tensor(
                h1[:].rearrange("c (q k) -> c q k", k=K),
                h1[:].rearrange("c (q k) -> c q k", k=K),
                bq[:].unsqueeze(2).to_broadcast([8, P, K]), ALU.add,
            )
            nc.scalar.activation(h1[:], h1[:], AF.Relu)
            h2 = p3big.tile([8, 2048], F32, tag="h2")
            for ck in range(4):
                sl = slice(ck * 512, (ck + 1) * 512)
                ps = ps_micro.tile([8, 512], F32, tag="psu")
                nc.tensor.matmul(ps[:], wn2t_s[:], h1[:, sl], start=True, stop=True)
                nc.scalar.activation(h2[:, sl], ps[:], AF.Relu,
                                     bias=wnb1_s[:], scale=1.0)
            wgt = p3big.tile([W, 2048], F32, tag="wgt")
            for ck in range(4):
                sl = slice(ck * 512, (ck + 1) * 512)
                ps = ps_micro.tile([W, 512], F32, tag="psu")
                nc.tensor.matmul(ps[:], wn3t_s[:], h2[:, sl], start=True, stop=True)
                nc.scalar.activation(wgt[:, sl], ps[:], AF.Relu,
                                     bias=wnb2_s[:], scale=1.0)
            s6 = p3big.tile([6, 2048], F32, tag="s6")
            for ck in range(4):
                sl = slice(ck * 512, (ck + 1) * 512)
                ps = ps_micro.tile([96, 512], F32, tag="psu")
                nc.tensor.matmul(ps[:], wrep16_s[:], wgt[:, sl], start=True, stop=True)
                tchunk = p3s.tile([96, 512], F32, tag="tchunk")
                nc.vector.tensor_tensor(tchunk[:], ps[:], Gt[0:96, sl], ALU.mult)
                ps2 = ps_micro.tile([6, 512], F32, tag="psu")
                nc.tensor.matmul(ps2[:], selmat_s[:], tchunk[:], start=True, stop=True)
                nc.scalar.activation(s6[:, sl], ps2[:], AF.Copy)
            koff = p3s.tile([6, P], F32, tag="koff")
            nc.vector.tensor_reduce(
                koff[:], s6[:].rearrange("c (q k) -> c q k", k=K),
                axis=AXX, op=ALU.add,
            )
            kws = p3s.tile([W, P], F32, tag="kws")
            nc.vector.tensor_reduce(
                kws[:], wgt[:].rearrange("c (q k) -> c q k", k=K),
                axis=AXX, op=ALU.add,
            )
            psU = ps_micro.tile([96, P], F32, tag="psu")
            nc.tensor.matmul(psU[:], rxt_s[:], qrows_s[:, ti * P:(ti + 1) * P],
                             start=True, stop=True)
            psR = ps_micro.tile([96, P], F32, tag="psu")
            nc.tensor.matmul(psR[:], wrep16_s[:], kws[:], start=True, stop=True)
            kwrep = p3s.tile([96, P], F32, tag="kwrep")
            nc.scalar.activation(kwrep[:], psR[:], AF.Copy)
            umul = p3s.tile([96, P], F32, tag="umul")
            nc.vector.tensor_tensor(umul[:], kwrep[:], psU[:], ALU.mult)
            psC = ps_micro.tile([6, P], F32, tag="psu")
            nc.tensor.matmul(psC[:], selmat_s[:], umul[:], start=True, stop=True)
            off = p3s.tile([6, P], F32, tag="off")
            nc.vector.tensor_tensor(off[:], koff[:], psC[:], ALU.subtract)
            nc.vector.tensor_tensor(off[:], off[:], fcb_s[:].to_broadcast([6, P]),
                                    ALU.add)
            if DEBUG:
                nc.sync.dma_start(t["dbg_off"][:, ti * P:(ti + 1) * P], off[:])
            nc.vector.tensor_tensor(
                warp_loc[:, ti * P:(ti + 1) * P], off[:],
                qq6[:, ti * P:(ti + 1) * P], ALU.add,
            )
        nc.sync.dma_start(t["warp_in"][:], warp_loc[:])
        nc.gpsimd.collective_compute(
            "AllGather", ALU.bypass, replica_groups=GROUPS,
            ins=[t["warp_in"][:].opt()], outs=[t["warp_out"][:].opt()],
        )

    # ------------------------------------------------------------------
    # P4: warped aug tables
    # ------------------------------------------------------------------
    with tc.tile_pool(name="p4", bufs=2) as p4:
        for wi in (0, 1):
            r0 = 32 * wi
            for blk in range(4):
                csl = slice(blk * NQ, (blk + 1) * NQ)
                rowsc = p4.tile([3, NQ], F32, tag="rowsc")
                nc.sync.dma_start(
                    rowsc[:], t["warp_out"][blk * 6 + 3 * wi: blk * 6 + 3 * wi + 3, :])
                rhi = p4.tile([3, NQ], BF16, tag="w4hi")
                nc.scalar.activation(rhi[:], rowsc[:], AF.Copy)
                rlo = p4.tile([3, NQ], BF16, tag="w4lo")
                nc.vector.tensor_tensor(rlo[:], rowsc[:], rhi[:], ALU.subtract)
                nc.sync.dma_start(augsb[r0:r0 + 3, csl], rhi[:])
                nc.sync.dma_start(augsb[r0 + 4:r0 + 7, csl], rhi[:])
                nc.sync.dma_start(augsb[r0 + 8:r0 + 11, csl], rlo[:])
                w2r = p4.tile([3, NQ], F32, tag="w2r")
                nc.vector.tensor_tensor(w2r[:], rowsc[:], rowsc[:], ALU.mult)
                nsq = p4.tile([1, NQ], F32, tag="wnsq")
                for ck in range(4):
                    sl = slice(ck * 512, (ck + 1) * 512)
                    psn = ps_micro.tile([1, 512], F32, tag="psu")
                    nc.tensor.matmul(psn[:], ones3_s[:], w2r[:, sl],
                                     start=True, stop=True)
                    nc.scalar.activation(nsq[:, sl], psn[:], AF.Copy)
                nhi = p4.tile([1, NQ], BF16, tag="w4nhi")
                nc.scalar.activation(nhi[:], nsq[:], AF.Copy)
                nlo = p4.tile([1, NQ], BF16, tag="w4nlo")
                nc.vector.tensor_tensor(nlo[:], nsq[:], nhi[:], ALU.subtract)
                nc.sync.dma_start(augsb[r0 + 3:r0 + 4, csl], nhi[:])
                nc.sync.dma_start(augsb[r0 + 7:r0 + 8, csl], nlo[:])

    # ------------------------------------------------------------------
    # P5: D3/D4 1-NN + final gather
    # ------------------------------------------------------------------
    with tc.tile_pool(name="p5s", bufs=2) as p5s:
        for ti in range(NT):
            idx12 = p5s.tile([P, 2], F32, tag="idx12")
            for wi in (0, 1):
                Vf, G = dmat_select(ti, 32 * wi)
                T8 = p5s.tile([P, 8], F32, tag="d34t8")
                nc.vector.max(out=T8[:], in_=Vf)
                idx1 = mask_extract(p5s, Vf, G, T8[:, 0:1], 1, "d34x")
                nc.vector.tensor_copy(idx12[:, wi:wi + 1], idx1[:])
            if DEBUG:
                nc.sync.dma_start(t["dbg_idx12"][:, ti * 2:(ti + 1) * 2], idx12[:])
            # per-group tables: groups 0..3 <- idx1, groups 4..7 <- idx2
            p1p = p5s.tile([P, 16], F32, tag="fgp1")
            nc.vector.tensor_copy(p1p[:], idx12[:, 0:1].to_broadcast([P, 16]))
            p2p = p5s.tile([P, 16], F32, tag="fgp2")
            nc.vector.tensor_copy(p2p[:], idx12[:, 1:2].to_broadcast([P, 16]))
            fg = gather8k(gtab[:], p1p[:], p5s, "fg", second=p2p[:])
            fgv = p5s.tile([P, P], F32, tag="fgv")
            nc.vector.tensor_copy(
                fgv[:], fg[:].rearrange("c (q jj) -> c q jj", jj=16)[:, :, 0])
            ps = ps_micro.tile([C, P], F32, tag="psu")
            nc.tensor.matmul(ps[:], sumsel_s[:], fgv[:], start=True, stop=True)
            nc.scalar.activation(outsb[:, ti * P:(ti + 1) * P], ps[:], AF.Copy)

    nc.sync.dma_start(t["out"][:], outsb[:])
    ctx.close()


# --------------------------------------------------------------------------
# host side
# --------------------------------------------------------------------------

_CACHE = {}


def _prep_inputs(inputs):
    xyz_1 = np.asarray(inputs["xyz_1"], np.float32)
    xyz_2 = np.asarray(inputs["xyz_2"], np.float32)
    feature_1 = np.asarray(inputs["feature_1"], np.float32)
    feature_2 = np.asarray(inputs["feature_2"], np.float32)
    conv1_w = np.asarray(inputs["conv1_w"], np.float32)
    fc_w = np.asarray(inputs["fc_w"], np.float32)
    fc3 = fc_w.reshape(6, OUT + 3, W)

    def bn2(x):
        return np.ascontiguousarray(np.asarray(x, np.float32).reshape(2, P).T)

    consts = dict(
        convwt=np.ascontiguousarray(conv1_w.T),
        bng=bn2(inputs["bn_gamma"]),
        bnb=bn2(inputs["bn_beta"]),
        bnm=bn2(inputs["bn_mean"]),
        bnv=bn2(inputs["bn_var"]),
        wn1t=np.ascontiguousarray(np.asarray(inputs["wn_w0"], np.float32).T),
        wn2t=np.ascontiguousarray(np.asarray(inputs["wn_w1"], np.float32).T),
        wn3t=np.ascontiguousarray(np.asarray(inputs["wn_w2"], np.float32).T),
        wnb0=np.asarray(inputs["wn_b0"], np.float32).reshape(8, 1),
        wnb1=np.asarray(inputs["wn_b1"], np.float32).reshape(8, 1),
        wnb2=np.asarray(inputs["wn_b2"], np.float32).reshape(W, 1),
        fcb=np.asarray(inputs["fc_b"], np.float32).reshape(6, 1),
    )
    rft = np.zeros((P, 192), np.float32)
    rfull = fc3[:, 3:, :]                       # [6, 256, 16]
    for h in range(2):
        blk = rfull[:, h * P:(h + 1) * P, :]    # [6, 128, 16]
        rft[:, h * 96:(h + 1) * 96] = blk.transpose(1, 0, 2).reshape(P, 96)
    consts["rft"] = rft
    consts["rxt"] = np.ascontiguousarray(
        fc3[:, 0:3, :].transpose(1, 0, 2).reshape(3, 96))
    selmat = np.zeros((96, 6), np.float32)
    for o in range(6):
        selmat[o * 16:(o + 1) * 16, o] = 1.0
    consts["selmat"] = selmat
    repmat = np.zeros((16, P), np.float32)
    for c in range(P):
        repmat[c % 16, c] = 1.0
    consts["repmat"] = repmat
    wrep = np.zeros((W, 96), np.float32)
    for o in range(6):
        wrep[:, o * 16:(o + 1) * 16] = np.eye(W, dtype=np.float32)
    consts["wrep16"] = wrep
    sumsel = np.zeros((P, C), np.float32)
    for c in range(C):
        sumsel[c, c] = 1.0
        sumsel[c + C, c] = 1.0
    consts["sumsel"] = sumsel
    consts["ident"] = np.eye(P, dtype=np.float32)
    gb = (float(CH) * (np.arange(NCH * 8) // 8)).astype(np.float32)
    consts["gbase"] = np.broadcast_to(gb, (P, NCH * 8)).copy()

    def pt128(rows):
        return np.ascontiguousarray(rows.T.reshape(64, P, 3).transpose(1, 0, 2))

    in_maps = []
    for core in range(8):
        b, r = core // 4, core % 4
        q0 = r * NQ
        x1, x2 = xyz_1[b], xyz_2[b]
        m = dict(consts)
        m.update(
            k1rows=np.ascontiguousarray(x1),
            k2rows=np.ascontiguousarray(x2),
            k1pt=pt128(x1),
            k2pt=pt128(x2),
            qrows=np.ascontiguousarray(x2[:, q0:q0 + NQ]),
            qpt=np.ascontiguousarray(
                x2[:, q0:q0 + NQ].T.reshape(NT, P, 3).transpose(1, 0, 2)),
            f1rows=np.ascontiguousarray(feature_1[b]),
            f2loc=np.ascontiguousarray(feature_2[b][:, q0:q0 + NQ]),
            f2rows=np.ascontiguousarray(feature_2[b]),
        )
        in_maps.append(m)
    return in_maps


def kernel(**inputs):
    if "nc" not in _CACHE:
        _CACHE["nc"] = build_program()
    nc = _CACHE["nc"]
    in_maps = _prep_inputs(inputs)
    res = run_bass_kernel_spmd(nc, in_maps, core_ids=list(range(8)))
    _CACHE["last_res"] = res
    out = np.zeros((B, N, C), np.float32)
    for core in range(8):
        b, r = core // 4, core % 4
        out[b, r * NQ:(r + 1) * NQ, :] = res.results[core]["out"].T
    if DEBUG:
        _CACHE["results"] = res.results
    return out



# revision 70
# speedup vs baseline: 2.7305x; 2.1667x over previous
"""Trainium2 Bass kernel for nn_Aligned_Feature_Aggregation.

Pipeline (B=2, N=8192, C=64, OUT=256, W=16, K=16):
  up1 = 3-NN inverse-distance interp of feature_1 (at xyz_1) onto xyz_2
  nf  = LeakyReLU(BN(conv1([up1; feature_2])))
  idx = 16-NN self KNN of xyz_2; WeightNet aggregation -> offsets [6, N]
  warped_{1,2} = xyz_2 + offsets; two 1-NN lookups against warped sets
  out = up1^T[idx1] + feature_2^T[idx2]   -> [B, N, 64]

Sharding: 8 cores = 2 batches x 4 query-quarters (2048 queries each).
Keys replicated per batch; AllGather (groups of 4) shares nf-derived fold
tables, up1 and the warped point sets.

Distance matrices are computed as Dt = 2*q.k - |k|^2 on the PE (fp32,
4-dim lifted contraction); bigger = nearer. Top-k per 1024-key chunk via
DVE max8 + max_index, then a mask-based merge (ties -> lowest index).
The [B,N,K,259]x[B,N,K,16] aggregation is folded through fc_w so only a
96-channel per-key table is gathered (R_all = fc3 . [k_xyz; nf]).
"""

import os
import numpy as np

import concourse.bass as bass
import concourse.mybir as mybir
import concourse.tile as tile_mod
from concourse.bass_utils import run_bass_kernel_spmd
from concourse.vector_clock import ScopedClock

F32 = mybir.dt.float32
BF16 = mybir.dt.bfloat16
U16 = mybir.dt.uint16
I16 = mybir.dt.int16
ALU = mybir.AluOpType
AF = mybir.ActivationFunctionType
AXX = mybir.AxisListType.X

B, N, C = 2, 8192, 64
OUT, W, K = 256, 16, 16
NQ = 2048            # queries per core
NT = 16              # query tiles of 128 per core
NCH = 8              # key chunks per tile
CH = 1024            # key chunk size
P = 128
BN_EPS = 1e-5
LEAKY = 0.1
NEG_BIG = -3.0e38
GROUPS = [[0, 1, 2, 3], [4, 5, 6, 7]]

DEBUG = bool(int(os.environ.get("BASS_KERNEL_DEBUG", "0")))


def _patch_tile_drain():
    """walrus in this env rejects >1 sem wait on the final SP drain; split."""
    if getattr(tile_mod.TileContext, "_drain_split_patched", False):
        return

    def _drain_and_barrier(self, tick_clock, wait_clock):
        nc = self.nc
        drain_inst = nc.sync.drain()
        wait_clock.add_sem_waits(
            drain_inst.ins, ScopedClock({None: tick_clock.global_clock})
        )
        si = drain_inst.ins.sync_info
        if si is not None and si.on_wait is not None and len(si.on_wait) > 1:
            waits = list(si.on_wait)
            si.on_wait = waits[:1]
            for w in waits[1:]:
                d2 = nc.sync.drain()
                d2.ins.sync_info = mybir.SyncInfo(on_wait=[w], on_update=[])
        nc.all_engine_barrier()
        assert self.sems is not None
        popped = nc._tile_sem_poison_stack.pop()
        assert popped is self._sem_poison
        nc.clear_and_free_semaphores(list(self.sems.allocated().values()))
        nc.all_engine_barrier()

    tile_mod.TileContext._drain_and_barrier = _drain_and_barrier
    tile_mod.TileContext._drain_split_patched = True


def build_program():
    _patch_tile_drain()
    nc = bass.Bass("TRN2", target_bir_lowering=False, debug=False)

    def din(name, shape):
        return nc.dram_tensor(name, shape, F32, kind="ExternalInput").ap()

    t = {}
    t["k1rows"] = din("k1rows", [3, N])
    t["k2rows"] = din("k2rows", [3, N])
    t["k1pt"] = din("k1pt", [P, 64, 3])
    t["k2pt"] = din("k2pt", [P, 64, 3])
    t["qrows"] = din("qrows", [3, NQ])
    t["qpt"] = din("qpt", [P, NT, 3])
    t["f1rows"] = din("f1rows", [C, N])
    t["f2loc"] = din("f2loc", [C, NQ])
    t["f2rows"] = din("f2rows", [C, N])
    t["convwt"] = din("convwt", [P, OUT])
    for nm in ("bng", "bnb", "bnm", "bnv"):
        t[nm] = din(nm, [P, 2])
    t["wn1t"] = din("wn1t", [3, 8])
    t["wn2t"] = din("wn2t", [8, 8])
    t["wn3t"] = din("wn3t", [8, W])
    t["wnb0"] = din("wnb0", [8, 1])
    t["wnb1"] = din("wnb1", [8, 1])
    t["wnb2"] = din("wnb2", [W, 1])
    t["rft"] = din("rft", [P, 192])
    t["rxt"] = din("rxt", [3, 96])
    t["repmat"] = din("repmat", [16, P])
    t["fcb"] = din("fcb", [6, 1])
    t["selmat"] = din("selmat", [96, 6])
    t["wrep16"] = din("wrep16", [W, 96])
    t["sumsel"] = din("sumsel", [P, C])
    t["ident"] = din("ident", [P, P])
    t["gbase"] = din("gbase", [P, NCH * 8])

    t["out"] = nc.dram_tensor("out", [C, NQ], F32, kind="ExternalOutput").ap()
    if DEBUG:
        for nm, shp in [
            ("dbg_idx3", [P, NT * 3]),
            ("dbg_up1t", [C, NQ]),
            ("dbg_nf", [P, 2 * NQ]),
            ("dbg_idx16", [P, NT * 16]),
            ("dbg_off", [6, NQ]),
            ("dbg_idx12", [P, NT * 2]),
        ]:
            t[nm] = nc.dram_tensor(nm, shp, F32, kind="ExternalOutput").ap()

    t["augrow"] = nc.dram_tensor("augrow_d", [4, N], BF16).ap()
    t["mega_in"] = nc.dram_tensor("mega_in", [P, NQ], F32).ap()
    t["mega_out"] = nc.dram_tensor("mega_out", [4 * P, NQ], F32).ap()
    t["up1_in"] = nc.dram_tensor("up1_in", [C, NQ], F32).ap()
    t["up1_out"] = nc.dram_tensor("up1_out", [4 * C, NQ], F32).ap()
    t["warp_in"] = nc.dram_tensor("warp_in", [6, NQ], F32).ap()
    t["warp_out"] = nc.dram_tensor("warp_out", [24, NQ], F32).ap()

    with tile_mod.TileContext(nc) as tc:
        _build(nc, tc, t)
    _split_excess_waits(nc)
    return nc


def _split_excess_waits(nc, limit=1):
    """walrus rejects >2 sync waits per instruction: hoist extras onto NoOps."""
    for bbh in nc.bb_map.values():
        inner = bbh.bb
        insts = inner.instructions
        out = []
        changed = False
        for inst in insts:
            si = inst.sync_info
            waits = list(si.on_wait) if si is not None and si.on_wait else []
            if len(waits) > limit:
                excess, keep = waits[:-limit], waits[-limit:]
                for j in range(0, len(excess), limit):
                    nop = mybir.InstNoOp(
                        name=f"{inst.name}-ws{j}", ins=[], outs=[])
                    nop.engine = inst.engine
                    nop.sync_info = mybir.SyncInfo(
                        on_wait=excess[j:j + limit], on_update=[])
                    out.append(nop)
                si.on_wait = keep
                changed = True
            out.append(inst)
        if changed:
            inner.instructions = out


def _build(nc, tc, t):
    import contextlib
    ctx = contextlib.ExitStack()


    # ------------------------------------------------------------------
    # persistent SBUF
    # ------------------------------------------------------------------
    persist = ctx.enter_context(tc.tile_pool(name="persist", bufs=1))
    # final gather table, bf16: rows 0:64 up1 (from AllGather), 64:128 f2
    gtab = persist.tile([P, N], BF16, tag="gtab")
    # Distance matmuls run in bf16 hi/lo split form: 14 contraction rows
    # computing (q_hi+q_lo).(k_hi+k_lo) exactly (fp32 parity).
    # Query aug rows, replicated at base partitions 0 and 32 to match the
    # two key-table slots (matmul requires equal base partitions):
    # rows 0:3 2q_hi, 3 -1, 4:7 2q_lo, 7 -1, 8:11 2q_hi, 11:14 2q_lo.
    qaugr = persist.tile([46, NQ], BF16, tag="qaugr")
    # Packed key-aug tables, SBUF resident (PE base partition must be
    # 0/32/64): rows r0+0:3 k_hi, r0+3 s_hi, r0+4:7 k_hi, r0+7 s_lo,
    # r0+8:11 k_lo, r0+11:14 k_lo.  r0=0: k1 (later warped1), r0=32: k2
    # (later warped2).
    augsb = persist.tile([46, N], BF16, tag="augsb")
    outsb = persist.tile([C, NQ], F32, tag="outsb")

    consts = ctx.enter_context(tc.tile_pool(name="consts", bufs=1))

    def load_const(name, shape):
        s = consts.tile(shape, F32, tag="c_" + name)
        nc.sync.dma_start(s[:], t[name][:])
        return s

    convwt_lo = consts.tile([C, OUT], F32, tag="c_convlo")
    nc.sync.dma_start(convwt_lo[:], t["convwt"][0:C, :])
    convwt_hi = consts.tile([C, OUT], F32, tag="c_convhi")
    nc.sync.dma_start(convwt_hi[:], t["convwt"][C:P, :])
    wn1t_s = load_const("wn1t", [3, 8])
    wn2t_s = load_const("wn2t", [8, 8])
    wn3t_s = load_const("wn3t", [8, W])
    wnb0_s = load_const("wnb0", [8, 1])
    wnb1_s = load_const("wnb1", [8, 1])
    wnb2_s = load_const("wnb2", [W, 1])
    rft_s = load_const("rft", [P, 192])
    rxt_s = load_const("rxt", [3, 96])
    repmat_s = load_const("repmat", [16, P])
    fcb_s = load_const("fcb", [6, 1])
    selmat_s = load_const("selmat", [96, 6])
    wrep16_s = load_const("wrep16", [W, 96])
    sumsel_s = load_const("sumsel", [P, C])
    ident_s = load_const("ident", [P, P])
    gbase_s = load_const("gbase", [P, NCH * 8])
    ones3_s = consts.tile([3, 1], F32, tag="c_ones3")
    nc.vector.memset(ones3_s[:], 1.0)
    ones64_s = consts.tile([1, C], F32, tag="c_ones64")
    nc.vector.memset(ones64_s[:], 1.0)
    qrows_s = load_const("qrows", [3, NQ])
    qpt_s = load_const("qpt", [P, NT, 3])

    # BN scale/bias: scale = g/sqrt(v+eps), bias = b - m*scale
    bn = consts.tile([P, 2, 4], F32, tag="bn")
    nc.sync.dma_start(bn[:, :, 0], t["bng"][:])
    nc.sync.dma_start(bn[:, :, 1], t["bnb"][:])
    nc.sync.dma_start(bn[:, :, 2], t["bnm"][:])
    nc.sync.dma_start(bn[:, :, 3], t["bnv"][:])
    bnsc = consts.tile([P, 2, 2], F32, tag="bnsc")
    tmpbn = consts.tile([P, 2], F32, tag="tmpbn")
    nc.vector.tensor_scalar_add(tmpbn[:], bn[:, :, 3], float(BN_EPS))
    nc.scalar.activation(tmpbn[:], tmpbn[:], AF.Sqrt)
    nc.vector.reciprocal(tmpbn[:], tmpbn[:])
    nc.vector.tensor_tensor(bnsc[:, :, 0], bn[:, :, 0], tmpbn[:], ALU.mult)
    nc.vector.tensor_tensor(tmpbn[:], bn[:, :, 2], bnsc[:, :, 0], ALU.mult)
    nc.vector.tensor_tensor(bnsc[:, :, 1], bn[:, :, 1], tmpbn[:], ALU.subtract)

    # qaug = [2*q; -1] -> bf16 hi/lo split, staged through a scoped pool
    with tc.tile_pool(name="qtmp", bufs=1) as qtmp:
        qaug = qtmp.tile([4, NQ], F32, tag="qaug")
        nc.vector.memset(qaug[:], -1.0)
        nc.scalar.mul(qaug[0:3, :], qrows_s[:], 2.0)
        qhi = qtmp.tile([4, NQ], BF16, tag="qhi")
        nc.scalar.activation(qhi[:], qaug[:], AF.Copy)
        qlo = qtmp.tile([4, NQ], BF16, tag="qlo")
        nc.vector.tensor_tensor(qlo[:], qaug[:], qhi[:], ALU.subtract)
        for b in (0, 32):
            nc.sync.dma_start(qaugr[b:b + 4, :], qhi[:])
            nc.sync.dma_start(qaugr[b + 4:b + 7, :], qlo[0:3, :])
            nc.sync.dma_start(qaugr[b + 7:b + 8, :], qhi[3:4, :])
            nc.sync.dma_start(qaugr[b + 8:b + 11, :], qhi[0:3, :])
            nc.sync.dma_start(qaugr[b + 11:b + 14, :], qlo[0:3, :])
    # qq6 = [q; q] for warped = offset + xyz2
    qq6 = persist.tile([6, NQ], F32, tag="qq6")
    nc.sync.dma_start(qq6[0:3, :], t["qrows"][:])
    nc.sync.dma_start(qq6[3:6, :], t["qrows"][:])

    # long-lived scratch pools
    mm_pool = ctx.enter_context(tc.tile_pool(name="mmp", bufs=3, space="PSUM"))
    ps_micro = ctx.enter_context(tc.tile_pool(name="psmicro", bufs=2, space="PSUM"))
    sel_pool = ctx.enter_context(tc.tile_pool(name="selp", bufs=3))
    tabp = ctx.enter_context(tc.tile_pool(name="tabp", bufs=3))

    # ------------------------------------------------------------------
    # helpers
    # ------------------------------------------------------------------
    def build_aug(pool, rows_src, pt_src_dram, r0, stage_hi, stage_lo):
        """bf16 hi/lo aug block at augsb[r0:r0+11] (see layout above)."""
        rows_f = pool.tile([3, N], F32, tag="augrows")
        nc.sync.dma_start(rows_f[:], rows_src)
        rhi = pool.tile([3, N], BF16, tag="aughi")
        nc.scalar.activation(rhi[:], rows_f[:], AF.Copy)
        rlo = pool.tile([3, N], BF16, tag="auglo")
        nc.vector.tensor_tensor(rlo[:], rows_f[:], rhi[:], ALU.subtract)
        nc.sync.dma_start(augsb[r0:r0 + 3, :], rhi[:])
        nc.sync.dma_start(augsb[r0 + 4:r0 + 7, :], rhi[:])
        nc.sync.dma_start(augsb[r0 + 8:r0 + 11, :], rlo[:])
        nc.sync.dma_start(augsb[r0 + 11:r0 + 14, :], rlo[:])
        ptt = pool.tile([P, 64, 3], F32, tag="augpt")
        nc.scalar.dma_start(ptt[:], pt_src_dram)
        sq = pool.tile([P, 64, 3], F32, tag="augsq")
        nc.vector.tensor_tensor(sq[:], ptt[:], ptt[:], ALU.mult)
        nsq = pool.tile([P, 64], F32, tag="augn")
        nc.vector.tensor_reduce(nsq[:], sq[:], axis=AXX, op=ALU.add)
        shi = pool.tile([P, 64], BF16, tag="augshi")
        nc.scalar.activation(shi[:], nsq[:], AF.Copy)
        slo = pool.tile([P, 64], BF16, tag="augslo")
        nc.vector.tensor_tensor(slo[:], nsq[:], shi[:], ALU.subtract)
        # s row col (g*128+p) <- s[p, g]; the partition-crossing scatter
        # needs HWDGE + a DRAM bounce (SBUF->SBUF can't balance the AP).
        for src, stage, rr in ((shi, stage_hi, 3), (slo, stage_lo, 7)):
            nc.sync.dma_start(
                stage.rearrange("one (g p) -> one p g", p=P), src[:])
            nc.sync.dma_start(augsb[r0 + rr:r0 + rr + 1, :], stage)

    def dmat_select(ti, r0):
        """f32r distance matmuls + per-chunk top8.  Returns (V, G) [P, 64]."""
        V = sel_pool.tile([P, NCH, 8], F32, tag="selV")
        Gu = sel_pool.tile([P, NCH, 8], U16, tag="selGu")
        lhs = qaugr[r0:r0 + 14, ti * P:(ti + 1) * P]
        for cki in range(NCH):
            ps = mm_pool.tile([P, CH], F32, tag="dmat")
            rhs = augsb[r0:r0 + 14, cki * CH:(cki + 1) * CH]
            for h in range(2):
                nc.tensor.matmul(
                    ps[:, h * 512:(h + 1) * 512], lhs,
                    rhs[:, h * 512:(h + 1) * 512], start=True, stop=True,
                )
            nc.vector.max(out=V[:, cki, :], in_=ps[:])
            nc.vector.max_index(out=Gu[:, cki, :], in_max=V[:, cki, :], in_values=ps[:])
        Vf = V[:].rearrange("p a b -> p (a b)")
        G = sel_pool.tile([P, NCH * 8], F32, tag="selGf")
        nc.vector.tensor_tensor(
            G[:], Gu[:].rearrange("p a b -> p (a b)"), gbase_s[:], ALU.add)
        return Vf, G

    def mask_extract(pool, Vf, G, ranks_ap, nk, tag):
        """idx[p, j] = G[p, pos(Vf == ranks[j])]; ties -> min index."""
        ncand = NCH * 8
        m = pool.tile([P, nk, ncand], mybir.dt.uint8, tag=tag + "m")
        nc.vector.tensor_tensor(
            m[:], Vf.unsqueeze(1).to_broadcast([P, nk, ncand]),
            ranks_ap.unsqueeze(2).to_broadcast([P, nk, ncand]), ALU.is_equal,
        )
        sel = pool.tile([P, nk, ncand], F32, tag=tag + "s")
        nc.vector.memset(sel[:], 65535.0)
        nc.vector.copy_predicated(
            sel[:], m[:], G[:].unsqueeze(1).to_broadcast([P, nk, ncand])
        )
        idx = pool.tile([P, nk], F32, tag=tag + "i")
        nc.vector.tensor_reduce(idx[:], sel[:], axis=AXX, op=ALU.min)
        return idx

    def transpose_pe(src_ap, m, tag):
        """[128, m<=128] -> PSUM [m, 128]"""
        ps = ps_micro.tile([m, P], F32, tag="psu")
        nc.tensor.matmul(ps[:], src_ap, ident_s[:], is_transpose=True)
        return ps

    def build_tab16(idxf16_ap, tag, second=None):
        """idxf16_ap [128, 16] f32 -> int16 ap_gather table [128, 128].

        Token i = q*16 + jj: unwrapped[i] = idxf16[q, jj] for every
        16-partition group. If `second` is given, groups 4..7 use it
        instead (per-group tables). Replication across the 8 groups is
        done on the PE via the block-replicate const `repmat`.
        """
        srcs = []
        for s_ap, stag in ((idxf16_ap, "a"), (second, "b")):
            if s_ap is None:
                srcs.append(None)
                continue
            tp = transpose_pe(s_ap, 16, tag + stag)
            tps = tabp.tile([16, P], F32, tag="ttps" + stag)
            nc.scalar.activation(tps[:], tp[:], AF.Copy)
            srcs.append(tps)
        tab = tabp.tile([P, P], U16, tag="ttabi")
        if second is None:
            tabP = ps_micro.tile([P, P], F32, tag="psu")
            nc.tensor.matmul(tabP[:], repmat_s[:], srcs[0][:],
                             start=True, stop=True)
            nc.vector.tensor_copy(tab[:], tabP[:])
        else:
            tabA = ps_micro.tile([64, P], F32, tag="psu")
            nc.tensor.matmul(tabA[:], repmat_s[:, 0:64], srcs[0][:],
                             start=True, stop=True)
            tabB = ps_micro.tile([64, P], F32, tag="psu")
            nc.tensor.matmul(tabB[:], repmat_s[:, 0:64], srcs[1][:],
                             start=True, stop=True)
            nc.vector.tensor_copy(tab[0:64, :], tabA[:])
            nc.vector.tensor_copy(tab[64:P, :], tabB[:])
        return tab



    def gather8k(data_ap, idx16f_ap, gpool, gtag, second=None, dtype=F32):
        """Gather [128ch, 2048tok] from a flat [128, 8192] table.

        idx16f in [0, 8192), used as global u16 indices directly.
        """
        if second is None:
            tab = build_tab16(idx16f_ap, gtag + "A")
        else:
            tab = build_tab16(idx16f_ap, gtag + "A", second=second)
        gA = gpool.tile([P, 2048], dtype, tag=gtag + "gA")
        for h in range(2):
            nc.gpsimd.indirect_copy(
                gA[:, h * 1024:(h + 1) * 1024], data_ap,
                tab[:, h * 64:(h + 1) * 64], True)
        return gA

    # ------------------------------------------------------------------
    # P0: key aug tables
    # ------------------------------------------------------------------
    with tc.tile_pool(name="p0", bufs=1) as p0:
        build_aug(p0, t["k1rows"][:], t["k1pt"][:], 0,
                  t["augrow"][0:1, :], t["augrow"][1:2, :])
        build_aug(p0, t["k2rows"][:], t["k2pt"][:], 32,
                  t["augrow"][2:3, :], t["augrow"][3:4, :])
    # f2 -> gtab rows 64:128 (flat bf16; gpsimd DMA casts f32 -> bf16)
    nc.gpsimd.dma_start(gtab[C:P, :], t["f2rows"][:])

    # ------------------------------------------------------------------
    # P1: D1 3-NN + upsample -> up1T
    # ------------------------------------------------------------------
    upool_cm = tc.tile_pool(name="upool", bufs=1)
    upool = upool_cm.__enter__()
    up1T = upool.tile([C, NQ], F32, tag="up1T")
    with tc.tile_pool(name="p1big", bufs=1) as p1big, \
         tc.tile_pool(name="p1w", bufs=1) as p1w, \
         tc.tile_pool(name="p1g", bufs=2) as p1g, \
         tc.tile_pool(name="p1s", bufs=2) as p1s:
        # flat gather table [128, 8192]: rows 0:3 xyz1, 64:128 feat1
        p1sb = p1big.tile([P, N], F32, tag="p1sb")
        nc.sync.dma_start(p1sb[0:3, :], t["k1rows"][:])
        nc.sync.dma_start(p1sb[C:P, :], t["f1rows"][:])

        def p1_front(ti):
            Vf, G = dmat_select(ti, 0)
            T8 = p1s.tile([P, 8], F32, tag="d1t8")
            nc.vector.max(out=T8[:], in_=Vf)
            idx3 = mask_extract(p1s, Vf, G, T8[:, 0:3], 3, "d1x")
            if DEBUG:
                nc.sync.dma_start(t["dbg_idx3"][:, ti * 3:(ti + 1) * 3], idx3[:])
            idx16p = p1s.tile([P, 16], F32, tag="d1pad")
            nc.vector.tensor_copy(idx16p[:, 0:3], idx3[:])
            nc.vector.tensor_copy(
                idx16p[:, 3:16], idx3[:, 0:1].to_broadcast([P, 13]))
            gout = gather8k(p1sb[:], idx16p[:], p1g, "p1g")
            return ti, gout

        def p1_back(st):
            ti, gout = st
            # token f = q*16 + jj (jj<3 used). rows 0:3 xyz1, 3:67 feat1
            gx = p1s.tile([3, P, 3], F32, tag="upg")
            nc.vector.tensor_tensor(
                gx[:], gout[0:3, :].rearrange("c (q jj) -> c q jj", jj=16)[:, :, 0:3],
                qrows_s[:, ti * P:(ti + 1) * P].unsqueeze(2).to_broadcast([3, P, 3]),
                ALU.subtract,
            )
            nc.vector.tensor_tensor(gx[:], gx[:], gx[:], ALU.mult)
            gxs = p1s.tile([3, P * 3], F32, tag="upgs")
            nc.vector.tensor_copy(gxs[:], gx[:].rearrange("c q jj -> c (q jj)"))
            psd = ps_micro.tile([1, P * 3], F32, tag="psu")
            nc.tensor.matmul(psd[:], ones3_s[:], gxs[:], start=True, stop=True)
            dist = p1s.tile([1, P, 3], F32, tag="updist")
            nc.scalar.activation(
                dist[:].rearrange("one q jj -> one (q jj)"), psd[:], AF.Sqrt)
            nc.vector.tensor_scalar_max(dist[:], dist[:], 1e-10)
            w3 = p1s.tile([1, P, 3], F32, tag="upw")
            nc.vector.reciprocal(w3[:], dist[:])
            wsum = p1s.tile([1, P], F32, tag="upws")
            nc.vector.tensor_reduce(wsum[:], w3[:], axis=AXX, op=ALU.add)
            nc.vector.reciprocal(wsum[:], wsum[:])
            nc.vector.tensor_tensor(
                w3[:], w3[:], wsum[:].unsqueeze(2).to_broadcast([1, P, 3]), ALU.mult)
            # replicate wn to 64 partitions via PE, padded to 16 jj (zeros)
            wn16 = p1w.tile([1, P, 16], F32, tag="wn16")
            nc.vector.memset(wn16[:], 0.0)
            nc.vector.tensor_copy(wn16[:, :, 0:3], w3[:])
            wrep = p1w.tile([P, 2048], F32, tag="uwrep")
            for ck in range(4):
                psx = ps_micro.tile([C, 512], F32, tag="psu")
                nc.tensor.matmul(
                    psx[:], ones64_s[:],
                    wn16[:].rearrange("one q jj -> one (q jj)")[:, ck * 512:(ck + 1) * 512],
                    start=True, stop=True,
                )
                nc.scalar.activation(wrep[C:P, ck * 512:(ck + 1) * 512], psx[:], AF.Copy)
            wf = p1w.tile([P, 2048], F32, tag="upwf")
            nc.vector.tensor_tensor(wf[C:P, :], gout[C:P, :], wrep[C:P, :], ALU.mult)
            nc.vector.tensor_reduce(
                up1T[:, ti * P:(ti + 1) * P],
                wf[C:P, :].rearrange("c (q jj) -> c q jj", jj=16),
                axis=AXX, op=ALU.add,
            )

        prev = None
        for ti in range(NT):
            cur = p1_front(ti)
            if prev is not None:
                p1_back(prev)
            prev = cur
        p1_back(prev)

    if DEBUG:
        nc.sync.dma_start(t["dbg_up1t"][:], up1T[:])

    # ------------------------------------------------------------------
    # P2: conv/BN/LeakyReLU -> nf; fold tables; AllGathers
    # ------------------------------------------------------------------
    if True:
        with tc.tile_pool(name="p2", bufs=1) as p2:
            f2loc_s = p2.tile([C, NQ], F32, tag="f2loc")
            nc.sync.dma_start(f2loc_s[:], t["f2loc"][:])
            nfsb = p2.tile([P, 2, NQ], F32, tag="nfsb")
            for h in range(2):
                for ck in range(4):
                    sl = slice(ck * 512, (ck + 1) * 512)
                    ps = ps_micro.tile([P, 512], F32, tag="psu")
                    nc.tensor.matmul(ps[:], convwt_lo[:, h * P:(h + 1) * P],
                                     up1T[:, sl], start=True, stop=False)
                    nc.tensor.matmul(ps[:], convwt_hi[:, h * P:(h + 1) * P],
                                     f2loc_s[:, sl], start=False, stop=True)
                    nc.scalar.activation(
                        nfsb[:, h, sl], ps[:], AF.Copy,
                        bias=0.0, scale=bnsc[:, h, 0:1],
                    )
                    # Copy ignores AP bias; add bias then LeakyReLU = max(x, 0.1x)
                    nc.vector.tensor_tensor(
                        nfsb[:, h, sl], nfsb[:, h, sl],
                        bnsc[:, h, 1:2].to_broadcast([P, 512]), ALU.add)
                    nc.vector.scalar_tensor_tensor(
                        nfsb[:, h, sl], nfsb[:, h, sl], LEAKY, nfsb[:, h, sl],
                        op0=ALU.mult, op1=ALU.max)
            if DEBUG:
                nc.sync.dma_start(
                    t["dbg_nf"][:], nfsb[:].rearrange("p a b -> p (a b)"))

            mega_loc = p2.tile([P, NQ], F32, tag="megaloc")
            nc.vector.memset(mega_loc[:], 0.0)
            for ck in range(4):
                sl = slice(ck * 512, (ck + 1) * 512)
                ps = ps_micro.tile([96, 512], F32, tag="psu")
                nc.tensor.matmul(ps[:], rft_s[:, 0:96], nfsb[:, 0, sl],
                                 start=True, stop=False)
                nc.tensor.matmul(ps[:], rft_s[:, 96:192], nfsb[:, 1, sl],
                                 start=False, stop=False)
                nc.tensor.matmul(ps[:], rxt_s[:], qrows_s[:, sl],
                                 start=False, stop=True)
                nc.scalar.activation(mega_loc[0:96, sl], ps[:], AF.Copy)
                ps2 = ps_micro.tile([8, 512], F32, tag="psu")
                nc.tensor.matmul(ps2[:], wn1t_s[:], qrows_s[:, sl],
                                 start=True, stop=True)
                nc.scalar.activation(mega_loc[96:104, sl], ps2[:], AF.Copy)

            nc.sync.dma_start(t["mega_in"][:], mega_loc[:])
            nc.gpsimd.collective_compute(
                "AllGather", ALU.bypass, replica_groups=GROUPS,
                ins=[t["mega_in"][:].opt()], outs=[t["mega_out"][:].opt()],
            )
            nc.sync.dma_start(t["up1_in"][:], up1T[:])
            nc.gpsimd.collective_compute(
                "AllGather", ALU.bypass, replica_groups=GROUPS,
                ins=[t["up1_in"][:].opt()], outs=[t["up1_out"][:].opt()],
            )
            for blk in range(4):
                nc.gpsimd.dma_start(
                    gtab[0:C, blk * NQ:(blk + 1) * NQ],
                    t["up1_out"][blk * C:(blk + 1) * C, :],
                )

    upool_cm.__exit__(None, None, None)

    # ------------------------------------------------------------------
    # P3: D2 16-NN + WeightNet aggregation -> warped
    # ------------------------------------------------------------------
    with tc.tile_pool(name="p3big", bufs=1) as p3big, \
         tc.tile_pool(name="p3g", bufs=2) as p3g, \
         tc.tile_pool(name="p3s", bufs=1) as p3s:
        megaT = p3big.tile([P, N], F32, tag="megaT")
        for blk in range(4):
            nc.sync.dma_start(
                megaT[:, blk * NQ:(blk + 1) * NQ],
                t["mega_out"][blk * P:(blk + 1) * P, :],
            )
        warp_loc = p3big.tile([6, NQ], F32, tag="warploc")

        def p3_front(ti):
            Vf, G = dmat_select(ti, 32)
            T16 = p3s.tile([P, 16], F32, tag="d2t16")
            nc.vector.max(out=T16[:, 0:8], in_=Vf)
            Vr = p3s.tile([P, NCH * 8], F32, tag="d2vr")
            nc.vector.match_replace(out=Vr[:], in_to_replace=T16[:, 0:8],
                                    in_values=Vf, imm_value=NEG_BIG)
            nc.vector.max(out=T16[:, 8:16], in_=Vr[:])
            idx16 = mask_extract(p3s, Vf, G, T16[:], 16, "d2x")
            if DEBUG:
                nc.sync.dma_start(t["dbg_idx16"][:, ti * 16:(ti + 1) * 16], idx16[:])
            Gt = gather8k(megaT[:], idx16[:], p3g, "d2g")
            return ti, Gt

        def p3_back(st):
            ti, Gt = st
            # token free layout: f = k*128 + q
            psb = ps_micro.tile([8, P], F32, tag="psu")
            nc.tensor.matmul(psb[:], wn1t_s[:], qrows_s[:, ti * P:(ti + 1) * P],
                             start=True, stop=True)
            bq = p3s.tile([8, P], F32, tag="bq")
            nc.vector.tensor_tensor(bq[:], wnb0_s[:].to_broadcast([8, P]), psb[:],
                                    ALU.subtract)
            h1 = p3big.tile([8, 2048], F32, tag="h1")
            nc.sync.dma_start(h1[:], Gt[96:104, :])
            nc.vector.tensor_tensor(
                h1[:].rearrange("c (q k) -> c q k", k=K),
                h1[:].rearrange("c (q k) -> c q k", k=K),
                bq[:].unsqueeze(2).to_broadcast([8, P, K]), ALU.add,
            )
            nc.scalar.activation(h1[:], h1[:], AF.Relu)
            h2 = p3big.tile([8, 2048], F32, tag="h2")
            for ck in range(4):
                sl = slice(ck * 512, (ck + 1) * 512)
                ps = ps_micro.tile([8, 512], F32, tag="psu")
                nc.tensor.matmul(ps[:], wn2t_s[:], h1[:, sl], start=True, stop=True)
                nc.scalar.activation(h2[:, sl], ps[:], AF.Relu,
                                     bias=wnb1_s[:], scale=1.0)
            wgt = p3big.tile([W, 2048], F32, tag="wgt")
            for ck in range(4):
                sl = slice(ck * 512, (ck + 1) * 512)
                ps = ps_micro.tile([W, 512], F32, tag="psu")
                nc.tensor.matmul(ps[:], wn3t_s[:], h2[:, sl], start=True, stop=True)
                nc.scalar.activation(wgt[:, sl], ps[:], AF.Relu,
                                     bias=wnb2_s[:], scale=1.0)
            s6 = p3big.tile([6, 2048], F32, tag="s6")
            for ck in range(4):
                sl = slice(ck * 512, (ck + 1) * 512)
                ps = ps_micro.tile([96, 512], F32, tag="psu")
                nc.tensor.matmul(ps[:], wrep16_s[:], wgt[:, sl], start=True, stop=True)
                tchunk = p3s.tile([96, 512], F32, tag="tchunk")
                nc.vector.tensor_tensor(tchunk[:], ps[:], Gt[0:96, sl], ALU.mult)
                ps2 = ps_micro.tile([6, 512], F32, tag="psu")
                nc.tensor.matmul(ps2[:], selmat_s[:], tchunk[:], start=True, stop=True)
                nc.scalar.activation(s6[:, sl], ps2[:], AF.Copy)
            koff = p3s.tile([6, P], F32, tag="koff")
            nc.vector.tensor_reduce(
                koff[:], s6[:].rearrange("c (q k) -> c q k", k=K),
                axis=AXX, op=ALU.add,
            )
            kws = p3s.tile([W, P], F32, tag="kws")
            nc.vector.tensor_reduce(
                kws[:], wgt[:].rearrange("c (q k) -> c q k", k=K),
                axis=AXX, op=ALU.add,
            )
            psU = ps_micro.tile([96, P], F32, tag="psu")
            nc.tensor.matmul(psU[:], rxt_s[:], qrows_s[:, ti * P:(ti + 1) * P],
                             start=True, stop=True)
            psR = ps_micro.tile([96, P], F32, tag="psu")
            nc.tensor.matmul(psR[:], wrep16_s[:], kws[:], start=True, stop=True)
            kwrep = p3s.tile([96, P], F32, tag="kwrep")
            nc.scalar.activation(kwrep[:], psR[:], AF.Copy)
            umul = p3s.tile([96, P], F32, tag="umul")
            nc.vector.tensor_tensor(umul[:], kwrep[:], psU[:], ALU.mult)
            psC = ps_micro.tile([6, P], F32, tag="psu")
            nc.tensor.matmul(psC[:], selmat_s[:], umul[:], start=True, stop=True)
            off = p3s.tile([6, P], F32, tag="off")
            nc.vector.tensor_tensor(off[:], koff[:], psC[:], ALU.subtract)
            nc.vector.tensor_tensor(off[:], off[:], fcb_s[:].to_broadcast([6, P]),
                                    ALU.add)
            if DEBUG:
                nc.sync.dma_start(t["dbg_off"][:, ti * P:(ti + 1) * P], off[:])
            nc.vector.tensor_tensor(
                warp_loc[:, ti * P:(ti + 1) * P], off[:],
                qq6[:, ti * P:(ti + 1) * P], ALU.add,
            )

        prev = None
        for ti in range(NT):
            cur = p3_front(ti)
            if prev is not None:
                p3_back(prev)
            prev = cur
        p3_back(prev)
        nc.sync.dma_start(t["warp_in"][:], warp_loc[:])
        nc.gpsimd.collective_compute(
            "AllGather", ALU.bypass, replica_groups=GROUPS,
            ins=[t["warp_in"][:].opt()], outs=[t["warp_out"][:].opt()],
        )

    # ------------------------------------------------------------------
    # P4: warped aug tables
    # ------------------------------------------------------------------
    with tc.tile_pool(name="p4", bufs=2) as p4:
        for wi in (0, 1):
            r0 = 32 * wi
            for blk in range(4):
                csl = slice(blk * NQ, (blk + 1) * NQ)
                rowsc = p4.tile([3, NQ], F32, tag="rowsc")
                nc.sync.dma_start(
                    rowsc[:], t["warp_out"][blk * 6 + 3 * wi: blk * 6 + 3 * wi + 3, :])
                rhi = p4.tile([3, NQ], BF16, tag="w4hi")
                nc.scalar.activation(rhi[:], rowsc[:], AF.Copy)
                rlo = p4.tile([3, NQ], BF16, tag="w4lo")
                nc.vector.tensor_tensor(rlo[:], rowsc[:], rhi[:], ALU.subtract)
                nc.sync.dma_start(augsb[r0:r0 + 3, csl], rhi[:])
                nc.sync.dma_start(augsb[r0 + 4:r0 + 7, csl], rhi[:])
                nc.sync.dma_start(augsb[r0 + 8:r0 + 11, csl], rlo[:])
                nc.sync.dma_start(augsb[r0 + 11:r0 + 14, csl], rlo[:])
                w2r = p4.tile([3, NQ], F32, tag="w2r")
                nc.vector.tensor_tensor(w2r[:], rowsc[:], rowsc[:], ALU.mult)
                nsq = p4.tile([1, NQ], F32, tag="wnsq")
                for ck in range(4):
                    sl = slice(ck * 512, (ck + 1) * 512)
                    psn = ps_micro.tile([1, 512], F32, tag="psu")
                    nc.tensor.matmul(psn[:], ones3_s[:], w2r[:, sl],
                                     start=True, stop=True)
                    nc.scalar.activation(nsq[:, sl], psn[:], AF.Copy)
                nhi = p4.tile([1, NQ], BF16, tag="w4nhi")
                nc.scalar.activation(nhi[:], nsq[:], AF.Copy)
                nlo = p4.tile([1, NQ], BF16, tag="w4nlo")
                nc.vector.tensor_tensor(nlo[:], nsq[:], nhi[:], ALU.subtract)
                nc.sync.dma_start(augsb[r0 + 3:r0 + 4, csl], nhi[:])
                nc.sync.dma_start(augsb[r0 + 7:r0 + 8, csl], nlo[:])

    # ------------------------------------------------------------------
    # P5: D3/D4 1-NN + final gather
    # ------------------------------------------------------------------
    with tc.tile_pool(name="p5s", bufs=2) as p5s, \
         tc.tile_pool(name="p5g", bufs=2) as p5g:

        def p5_front(ti):
            idx12 = p5s.tile([P, 2], F32, tag="idx12")
            for wi in (0, 1):
                Vf, G = dmat_select(ti, 32 * wi)
                T8 = p5s.tile([P, 8], F32, tag="d34t8")
                nc.vector.max(out=T8[:], in_=Vf)
                idx1 = mask_extract(p5s, Vf, G, T8[:, 0:1], 1, "d34x")
                nc.vector.tensor_copy(idx12[:, wi:wi + 1], idx1[:])
            if DEBUG:
                nc.sync.dma_start(t["dbg_idx12"][:, ti * 2:(ti + 1) * 2], idx12[:])
            # per-group tables: groups 0..3 <- idx1, groups 4..7 <- idx2
            p1p = p5s.tile([P, 16], F32, tag="fgp1")
            nc.vector.tensor_copy(p1p[:], idx12[:, 0:1].to_broadcast([P, 16]))
            p2p = p5s.tile([P, 16], F32, tag="fgp2")
            nc.vector.tensor_copy(p2p[:], idx12[:, 1:2].to_broadcast([P, 16]))
            fg = gather8k(gtab[:], p1p[:], p5g, "fg", second=p2p[:],
                          dtype=BF16)
            return ti, fg

        def p5_back(st):
            ti, fg = st
            fgv = p5s.tile([P, P], F32, tag="fgv")
            nc.vector.tensor_copy(
                fgv[:], fg[:].rearrange("c (q jj) -> c q jj", jj=16)[:, :, 0])
            ps = ps_micro.tile([C, P], F32, tag="psu")
            nc.tensor.matmul(ps[:], sumsel_s[:], fgv[:], start=True, stop=True)
            nc.scalar.activation(outsb[:, ti * P:(ti + 1) * P], ps[:], AF.Copy)

        prev = None
        for ti in range(NT):
            cur = p5_front(ti)
            if prev is not None:
                p5_back(prev)
            prev = cur
        p5_back(prev)

    nc.sync.dma_start(t["out"][:], outsb[:])
    ctx.close()


# --------------------------------------------------------------------------
# host side
# --------------------------------------------------------------------------

_CACHE = {}


def _prep_inputs(inputs):
    xyz_1 = np.asarray(inputs["xyz_1"], np.float32)
    xyz_2 = np.asarray(inputs["xyz_2"], np.float32)
    feature_1 = np.asarray(inputs["feature_1"], np.float32)
    feature_2 = np.asarray(inputs["feature_2"], np.float32)
    conv1_w = np.asarray(inputs["conv1_w"], np.float32)
    fc_w = np.asarray(inputs["fc_w"], np.float32)
    fc3 = fc_w.reshape(6, OUT + 3, W)

    def bn2(x):
        return np.ascontiguousarray(np.asarray(x, np.float32).reshape(2, P).T)

    consts = dict(
        convwt=np.ascontiguousarray(conv1_w.T),
        bng=bn2(inputs["bn_gamma"]),
        bnb=bn2(inputs["bn_beta"]),
        bnm=bn2(inputs["bn_mean"]),
        bnv=bn2(inputs["bn_var"]),
        wn1t=np.ascontiguousarray(np.asarray(inputs["wn_w0"], np.float32).T),
        wn2t=np.ascontiguousarray(np.asarray(inputs["wn_w1"], np.float32).T),
        wn3t=np.ascontiguousarray(np.asarray(inputs["wn_w2"], np.float32).T),
        wnb0=np.asarray(inputs["wn_b0"], np.float32).reshape(8, 1),
        wnb1=np.asarray(inputs["wn_b1"], np.float32).reshape(8, 1),
        wnb2=np.asarray(inputs["wn_b2"], np.float32).reshape(W, 1),
        fcb=np.asarray(inputs["fc_b"], np.float32).reshape(6, 1),
    )
    rft = np.zeros((P, 192), np.float32)
    rfull = fc3[:, 3:, :]                       # [6, 256, 16]
    for h in range(2):
        blk = rfull[:, h * P:(h + 1) * P, :]    # [6, 128, 16]
        rft[:, h * 96:(h + 1) * 96] = blk.transpose(1, 0, 2).reshape(P, 96)
    consts["rft"] = rft
    consts["rxt"] = np.ascontiguousarray(
        fc3[:, 0:3, :].transpose(1, 0, 2).reshape(3, 96))
    selmat = np.zeros((96, 6), np.float32)
    for o in range(6):
        selmat[o * 16:(o + 1) * 16, o] = 1.0
    consts["selmat"] = selmat
    repmat = np.zeros((16, P), np.float32)
    for c in range(P):
        repmat[c % 16, c] = 1.0
    consts["repmat"] = repmat
    wrep = np.zeros((W, 96), np.float32)
    for o in range(6):
        wrep[:, o * 16:(o + 1) * 16] = np.eye(W, dtype=np.float32)
    consts["wrep16"] = wrep
    sumsel = np.zeros((P, C), np.float32)
    for c in range(C):
        sumsel[c, c] = 1.0
        sumsel[c + C, c] = 1.0
    consts["sumsel"] = sumsel
    consts["ident"] = np.eye(P, dtype=np.float32)
    gb = (float(CH) * (np.arange(NCH * 8) // 8)).astype(np.float32)
    consts["gbase"] = np.broadcast_to(gb, (P, NCH * 8)).copy()

    def pt128(rows):
        return np.ascontiguousarray(rows.T.reshape(64, P, 3).transpose(1, 0, 2))

    in_maps = []
    for core in range(8):
        b, r = core // 4, core % 4
        q0 = r * NQ
        x1, x2 = xyz_1[b], xyz_2[b]
        m = dict(consts)
        m.update(
            k1rows=np.ascontiguousarray(x1),
            k2rows=np.ascontiguousarray(x2),
            k1pt=pt128(x1),
            k2pt=pt128(x2),
            qrows=np.ascontiguousarray(x2[:, q0:q0 + NQ]),
            qpt=np.ascontiguousarray(
                x2[:, q0:q0 + NQ].T.reshape(NT, P, 3).transpose(1, 0, 2)),
            f1rows=np.ascontiguousarray(feature_1[b]),
            f2loc=np.ascontiguousarray(feature_2[b][:, q0:q0 + NQ]),
            f2rows=np.ascontiguousarray(feature_2[b]),
        )
        in_maps.append(m)
    return in_maps


def kernel(**inputs):
    if "nc" not in _CACHE:
        _CACHE["nc"] = build_program()
    nc = _CACHE["nc"]
    in_maps = _prep_inputs(inputs)
    res = run_bass_kernel_spmd(nc, in_maps, core_ids=list(range(8)))
    _CACHE["last_res"] = res
    out = np.zeros((B, N, C), np.float32)
    for core in range(8):
        b, r = core // 4, core % 4
        out[b, r * NQ:(r + 1) * NQ, :] = res.results[core]["out"].T
    if DEBUG:
        _CACHE["results"] = res.results
    return out

